# revision 40
# baseline (speedup 1.0000x reference)
"""Blockwise-parallel transformer attention on 8 TRN2 NeuronCores.

Reference computation (per batch b):
    k = x@Wk + bk ; v = x@Wv + bv            (from ORIGINAL x, layer-invariant)
    h = x
    6x (shared weights):
        q = h@Wq + bq
        P = softmax(q k^T / 8)
        attn = (P @ v) / sqrt(512)
        ff = relu(attn@W1 + b1)@W2 + b2
        h = LN2(LN1(h + ff))

Sharding: 8 cores = 4 batches x 2 query-halves. Each core computes full
k/v for its batch (once), then processes its 1024-query slice through all
6 layers with zero cross-core traffic.

On-chip layout is fully transposed (feature dim on partitions, tokens on
the free axis); the host feeds x^T so the device never transposes.

Key algebraic move: scores = (h Wq + bq) k^T = h (k Wq^T)^T + bq k^T.
k is layer-invariant, so k' = k Wq^T is computed ONCE at setup and the
per-layer q projection disappears entirely; bq folds into the exp bias
via ck = (k@bq)/8. The residual add rides in the PSUM->SBUF copy
(DVE tensor_tensor) instead of an identity matmul.

fp8 numerics: fp8 weights are host-prescaled x64 (values ~0.02 would
otherwise sit in the fp8e4m3 subnormal range) and rescaled by 1/64 in
the PSUM->SBUF copies; v is stored x16 for the same reason. ff1 runs
fp8 DoubleRow on attn/1024 with W1*64.

Trivial-path (g=1,b=0,zero biases) specializations: LN2(LN1(t)) fuses
to (t-mu1)*alpha; softmax exps merge pairwise over 2-bank PSUM tiles
(ACT cost is per-column, so this nearly halves the exp chain); the
softmax-denominator partial sums ride the exp-wait slots of the scores
stream, one step ahead of their consumer; the LN statistics are derived
algebraically from (r, hf8) alone -- colsum(t) = w2c@r since the LN
output is zero-mean, colsum(t^2) = r^T(W2 W2^T)r + 2r^T(W2 h) + D since
it is unit-variance (layer 0 uses precomputed sum(x), sum(x^2)) -- so
they run interleaved in the scores stream two steps ahead of the
normalize, which itself rides chunk-pair-wise behind the ff2 residual
adds. Per-iteration PE order: attn(s), ff1(s), scores(s+1) x8 groups
(with denominator partials + LN stats of s in the gaps), ff2(s) with
inline normalize. The final step emits ff2+normalize+store in
half-windows so the tail drains incrementally.
"""

import sys

if "/opt/trn_rl_repo" not in sys.path:
    sys.path.insert(0, "/opt/trn_rl_repo")

import numpy as np
import ml_dtypes

import concourse.bass as bass
import concourse.mybir as mybir
import concourse.tile as tile
from concourse import bacc
import concourse.hw_specs as _hw_specs


def _restrict_act_tables():
    """All activation functions this kernel uses (exp, ln, relu, copy)
    live in the natural_log_exp_and_others table set. Left to its own
    devices the table-load pass alternates between exp_and_others and the
    ln set (~49 reloads x 1.5us of ACT time per run); restricting the
    offered sets collapses that to a single load. Dict order is preserved
    so act_func_set_id stays aligned with act_info.json."""
    if getattr(_hw_specs, "_act_tables_restricted", False):
        return
    orig = _hw_specs.get_activation_tables

    def restricted(arch):
        tables = orig(arch)
        return {
            name: (fns if name == "natural_log_exp_and_others" else set())
            for name, fns in tables.items()
        }

    _hw_specs.get_activation_tables = restricted
    bacc.get_activation_tables = restricted
    _hw_specs._act_tables_restricted = True


_restrict_act_tables()
from concourse.bass_utils import run_bass_kernel_spmd
from concourse.masks import make_identity

F32 = mybir.dt.float32
BF16 = mybir.dt.bfloat16
F8 = mybir.dt.float8e4
DR = mybir.MatmulPerfMode.DoubleRow
EXP = mybir.ActivationFunctionType.Exp
LN_ = mybir.ActivationFunctionType.Ln
RELU = mybir.ActivationFunctionType.Relu
IDENT = mybir.ActivationFunctionType.Identity
ADD = mybir.AluOpType.add
SUB = mybir.AluOpType.subtract
MULT = mybir.AluOpType.mult

B, S, D, HID, L = 4, 2048, 512, 64, 6
EPS = 1e-5
P = 128
WS = 64.0   # fp8 weight prescale (host x64, on-chip 1/64)
VS = 16.0   # v prescale


def build(S=S, SQ=S // 2, D=D, HID=HID, L=L, trivial_ln=False, trivial_bias=False):
    """Build + compile the per-core Bass program (same program on all 8 cores)."""
    C = D // P          # feature-dim 128-chunks (4)
    MK = S // P         # key-token 128-chunks (16)
    FK = min(512, S)    # key free-dim tile
    NK = S // FK
    FQ = min(512, SQ)   # query free-dim tile
    NQ = SQ // FQ
    scale_attn = 1.0 / float(np.sqrt(HID))
    scale_out = 1.0 / float(np.sqrt(D))

    nc = bacc.Bacc("TRN2", target_bir_lowering=False, debug=False)

    # ---- DRAM I/O (per core) ----
    xt = nc.dram_tensor("xt", (C, P, S), F8, kind="ExternalInput")
    xq = nc.dram_tensor("xq", (C, P, SQ), BF16, kind="ExternalInput")
    xq8 = nc.dram_tensor("xq8", (C, P, SQ), F8, kind="ExternalInput")
    wqT = nc.dram_tensor("wq", (C, P, D), F8, kind="ExternalInput")   # Wq^T x64
    wk = nc.dram_tensor("wk", (C, P, D), F8, kind="ExternalInput")    # Wk x64
    wv = nc.dram_tensor("wv", (C, P, D), F8, kind="ExternalInput")    # Wv x64
    w1 = nc.dram_tensor("w1", (C, P, HID), F8, kind="ExternalInput")  # W1 x64
    w2 = nc.dram_tensor("w2", (HID, D), BF16, kind="ExternalInput")
    w2c = nc.dram_tensor("w2c", (HID, 1), BF16, kind="ExternalInput")
    w2t = nc.dram_tensor("w2t", (C, P, HID), F8, kind="ExternalInput")  # W2^T x64
    g64 = nc.dram_tensor("g64", (HID, HID), BF16, kind="ExternalInput")  # W2 W2^T
    bqc = nc.dram_tensor("bqc", (C, P, 1), F8, kind="ExternalInput")
    bk = nc.dram_tensor("bk", (P, C), F32, kind="ExternalInput")
    bv = nc.dram_tensor("bv", (1, D), F32, kind="ExternalInput")      # bv x16
    b1d = nc.dram_tensor("b1d", (HID, 1), F32, kind="ExternalInput")
    b2r = nc.dram_tensor("b2r", (1, D), BF16, kind="ExternalInput")
    g1d = nc.dram_tensor("g1d", (P, C), F32, kind="ExternalInput")
    be1d = nc.dram_tensor("be1d", (P, C), F32, kind="ExternalInput")
    g2d = nc.dram_tensor("g2d", (P, C), F32, kind="ExternalInput")
    be2d = nc.dram_tensor("be2d", (P, C), F32, kind="ExternalInput")
    out = nc.dram_tensor("out", (C, P, SQ), BF16, kind="ExternalOutput")

    # PSUM bank budget (8 banks):
    #  trivial: psA x2 [P,FQ] (attn/ff) + psP x3 of 2 banks (score pairs,
    #           LN stats and softmax denominator share psP's rotation)
    #  general: psA x6 + psS x2 (baseline layout)
    nA = 2 if trivial_bias else 6
    with tile.TileContext(nc) as tc:
        with (
            tc.tile_pool(name="const", bufs=1) as cons,
            tc.tile_pool(name="big", bufs=1) as big,
            tc.tile_pool(name="vec", bufs=2) as vecp,
            tc.tile_pool(name="psA", bufs=nA, space="PSUM") as psA,
            tc.tile_pool(name="psS", bufs=2, space="PSUM") as psS,
            tc.tile_pool(name="psP", bufs=3, space="PSUM") as psP,
        ):
            def stat_tile(tw):
                if trivial_bias:
                    return psP.tile([1, tw], F32, tag="pp", name="pstat")
                return psS.tile([1, tw], F32, tag="stat", name="pstat")

            # ---- persistent SBUF ----
            wqT_sb = cons.tile([P, C, D], F8)
            wk_sb = cons.tile([P, C, D], F8)
            wv_sb = cons.tile([P, C, D], F8)
            w1_sb = cons.tile([P, C, HID], F8)
            w2_sb = cons.tile([HID, D], BF16)
            w2c_sb = cons.tile([HID, 1], BF16)
            w2t_sb = cons.tile([P, C, HID], F8)
            g64_sb = cons.tile([HID, HID], BF16)
            e_sb = cons.tile([HID, SQ], BF16)   # r*(Gr + 2 W2 h) for var stats
            xsum_sb = cons.tile([1, SQ], BF16)  # per-token sum(x) (layer 0)
            xsqs_sb = cons.tile([1, SQ], BF16)  # per-token sum(x^2) (layer 0)
            onep_sb = cons.tile([1, 1], F32)    # 1 + eps + eps^2
            bqc_sb = cons.tile([P, C], F8)
            bk_sb = cons.tile([P, C], F32)
            bv_sb = cons.tile([1, D], F32)
            bv_bc = cons.tile([P, D], F32)
            b1_sb = cons.tile([HID, 1], F32)
            b2r_sb = cons.tile([1, D], BF16)
            g1_sb = cons.tile([P, C], F32)
            be1_sb = cons.tile([P, C], F32)
            g2_sb = cons.tile([P, C], F32)
            be2_sb = cons.tile([P, C], F32)
            ones_bf = cons.tile([P, 1], BF16)
            ones2_f8 = cons.tile([P, 2, 16], F8)  # pair-dim stride must be 16B
            ones_row = cons.tile([1, SQ], BF16)
            eps_sb = cons.tile([1, 1], F32)
            eps2_sb = cons.tile([1, 1], F32)
            ident_sb = cons.tile([P, P], BF16)
            ck_sb = cons.tile([P, MK], F32)   # exp bias: (k @ bq)/8 per key token

            k_sb = cons.tile([P, C, S], F8)       # k^T (fp8)
            kq_sb = cons.tile([P, C, S], F8)      # k'^T = Wq k^T (fp8)
            v_sb = cons.tile([P, MK, D], F8)      # v natural x16 (fp8)
            vbf_sb = None
            if not trivial_bias:
                vbf_sb = cons.tile([P, MK, D], BF16)
            h_sb = cons.tile([P, C, SQ], BF16)    # h^T (residual stream)
            hf8_sb = cons.tile([P, C, SQ], F8)    # h^T in fp8 for the scores matmul
            attn8_sb = cons.tile([P, C, SQ], F8)  # attn^T / (WS*VS) in fp8
            r_sb = cons.tile([HID, SQ], BF16)     # relu(ffn hidden)
            t_sb = cons.tile([P, C, SQ], BF16)    # residual pre-LN
            tsq_sb = cons.tile([P, C, SQ], BF16)
            stw_sb = None
            if not trivial_ln:
                stw_sb = cons.tile([P, 2, SQ], BF16)  # [sum(t), sum(t^2)]
            hout_sb = cons.tile([P, C, SQ], BF16)  # final-layer output
            recip_bc = cons.tile([P, SQ], F32)
            mu1_bc = cons.tile([P, SQ], BF16)
            rstd1_bc = cons.tile([P, SQ], BF16)
            mu2_bc = cons.tile([P, SQ], BF16)
            rstd2_bc = cons.tile([P, SQ], BF16)

            # fp8 P is safe only with zero biases (logits stay in ~[-3,3]);
            # the general path keeps bf16 P and standard matmuls
            P_dt = F8 if trivial_bias else BF16
            P_sb = cons.tile([P, MK, SQ], P_dt)  # exp(scores^T)

            xt_sb = big.tile([P, C, S], F8, tag="bigshare")

            # ---- constants first so the PE warmup isn't gated on queue
            # drain, then inputs spread across engine queues (each dma_start
            # costs ~600ns of issue time on its queue) ----
            nc.vector.memset(ones_bf[:], 1.0)
            nc.vector.memset(ones2_f8[:], 1.0)
            nc.vector.memset(eps_sb[:], EPS)
            nc.vector.memset(eps2_sb[:], EPS * EPS)
            nc.vector.memset(onep_sb[:], 1.0 + EPS + EPS * EPS)
            if not trivial_bias:
                nc.vector.memset(ones_row[:], 1.0)
            make_identity(nc, ident_sb[:])

            for c in range(C):
                nc.sync.dma_start(wk_sb[:, c, :], wk[c, :, :])
            for c in range(C):
                nc.sync.dma_start(xt_sb[:, c, 0:S // 2], xt[c, :, 0:S // 2])
            for c in range(C):
                nc.sync.dma_start(xt_sb[:, c, S // 2:S], xt[c, :, S // 2:S])
            nc.gpsimd.dma_start(bk_sb[:], bk[:, :])
            for c in range(C):
                nc.gpsimd.dma_start(wqT_sb[:, c, :], wqT[c, :, :])
            for c in range(C):
                nc.gpsimd.dma_start(wv_sb[:, c, :], wv[c, :, :])
            nc.gpsimd.dma_start(w1_sb[:], w1[:, :, :].rearrange("c p d -> p c d"))
            nc.gpsimd.dma_start(w2_sb[:], w2[:, :])
            nc.gpsimd.dma_start(w2c_sb[:], w2c[:, :])
            nc.gpsimd.dma_start(w2t_sb[:], w2t[:, :, :].rearrange("c p d -> p c d"))
            nc.gpsimd.dma_start(g64_sb[:], g64[:, :])
            for c in range(C):
                nc.scalar.dma_start(hf8_sb[:, c, :], xq8[c, :, :])
            nc.gpsimd.dma_start(b1_sb[:], b1d[:, :])
            if not trivial_bias:
                nc.gpsimd.dma_start(bqc_sb[:], bqc[:, :, 0].rearrange("c p -> p c"))
                nc.gpsimd.dma_start(bv_sb[:], bv[:, :])
                nc.gpsimd.dma_start(b2r_sb[:], b2r[:, :])
            if not trivial_ln:
                nc.gpsimd.dma_start(g1_sb[:], g1d[:, :])
                nc.gpsimd.dma_start(be1_sb[:], be1d[:, :])
                nc.gpsimd.dma_start(g2_sb[:], g2d[:, :])
                nc.gpsimd.dma_start(be2_sb[:], be2d[:, :])
            if not trivial_bias:
                nc.gpsimd.partition_broadcast(bv_bc[:], bv_sb[0:1, :])

            # HAM warmup: keep the PE busy while the input DMAs land so the
            # first real matmuls run at full clock
            wu = psA.tile([P, P], F32, tag="main")
            for _ in range(80):
                nc.tensor.matmul(wu[:], ident_sb[:], ident_sb[:],
                                 start=True, stop=True)

            # ---- k^T = (Wk^T x^T)/WS + bk ;  k'^T = (Wq k^T)/WS ----
            def k_tile(nk):
                for c in range(C):
                    ps = psA.tile([P, FK], F32, tag="main")
                    for t2 in range(C // 2):
                        nc.tensor.matmul(
                            ps[:],
                            wk_sb[:, 2 * t2:2 * t2 + 2, c * P:(c + 1) * P],
                            xt_sb[:, 2 * t2:2 * t2 + 2, nk * FK:(nk + 1) * FK],
                            start=(t2 == 0),
                            stop=(t2 == C // 2 - 1),
                            perf_mode=DR,
                        )
                    nc.scalar.activation(
                        k_sb[:, c, nk * FK:(nk + 1) * FK], ps[:], IDENT,
                        bias=bk_sb[:, c:c + 1], scale=1.0 / WS,
                    )

            def kq_tile(nk):
                for c in range(C):
                    ps = psA.tile([P, FK], F32, tag="main")
                    for t2 in range(C // 2):
                        nc.tensor.matmul(
                            ps[:],
                            wqT_sb[:, 2 * t2:2 * t2 + 2, c * P:(c + 1) * P],
                            k_sb[:, 2 * t2:2 * t2 + 2, nk * FK:(nk + 1) * FK],
                            start=(t2 == 0),
                            stop=(t2 == C // 2 - 1),
                            perf_mode=DR,
                        )
                    if c % 2 == 0:
                        nc.scalar.activation(
                            kq_sb[:, c, nk * FK:(nk + 1) * FK], ps[:], IDENT,
                            scale=1.0 / WS)
                    else:
                        nc.vector.tensor_scalar_mul(
                            kq_sb[:, c, nk * FK:(nk + 1) * FK], ps[:], 1.0 / WS)

            k_tile(0)
            k_tile(1)
            kq_tile(0)
            k_tile(2)
            kq_tile(1)
            k_tile(3)
            kq_tile(2)
            kq_tile(3)

            # h (bf16) is needed first by the layer-0 x-stats (~20us in);
            # its DMA issues late so it doesn't compete with the k-path
            for c in range(C):
                nc.gpsimd.dma_start(h_sb[:, c, :], xq[c, :, :])



            # ---- ck = (k @ bq) * scale_attn  (exp bias; layer-invariant) ----
            for mk in range(MK) if not trivial_bias else []:
                ps = psS.tile([P, 1], F32, tag="stat")
                for c in range(C):
                    nc.tensor.matmul(
                        ps[:],
                        k_sb[:, c, mk * P:(mk + 1) * P],
                        bqc_sb[:, c:c + 1],
                        start=(c == 0),
                        stop=(c == C - 1),
                    )
                nc.vector.tensor_scalar_mul(ck_sb[:, mk:mk + 1], ps[:], scale_attn)

            # ---- v = (x@Wv)*VS/WS + bv*VS ----
            for mk in range(MK):
                ps = psA.tile([P, D], F32, tag="main")
                for t2 in range(C // 2):
                    nc.tensor.matmul(
                        ps[:],
                        xt_sb[:, 2 * t2:2 * t2 + 2, mk * P:(mk + 1) * P],
                        wv_sb[:, 2 * t2:2 * t2 + 2, :],
                        start=(t2 == 0),
                        stop=(t2 == C // 2 - 1),
                        perf_mode=DR,
                    )
                if trivial_bias:
                    # bv == 0 in the trivial path: plain scaled copy
                    nc.vector.tensor_scalar_mul(v_sb[:, mk, :], ps[:], VS / WS)
                else:
                    nc.vector.scalar_tensor_tensor(
                        v_sb[:, mk, :], ps[:], VS / WS, bv_bc[:], MULT, ADD)
                    nc.vector.scalar_tensor_tensor(
                        vbf_sb[:, mk, :], ps[:], VS / WS, bv_bc[:], MULT, ADD)

            # ---- per-chunk pipeline pieces ----
            def scores_group(nq, mg):
                # one pair of key chunks shares a 2-bank PSUM tile -> one
                # [128, 2*FQ] exp per pair (ACT cost is per-column); the
                # 3-deep psP rotation lets the matmuls run ahead of the
                # exp chain instead of pacing to it
                ts = slice(nq * FQ, (nq + 1) * FQ)
                pp = psP.tile([P, 2 * FQ], F32, tag="pp")
                for half in range(2):
                    mk = 2 * mg + half
                    for t2 in range(C // 2):
                        nc.tensor.matmul(
                            pp[:, half * FQ:(half + 1) * FQ],
                            kq_sb[:, 2 * t2:2 * t2 + 2,
                                  mk * P:(mk + 1) * P],
                            hf8_sb[:, 2 * t2:2 * t2 + 2, ts],
                            start=(t2 == 0),
                            stop=(t2 == C // 2 - 1),
                            perf_mode=DR,
                        )
                nc.scalar.activation(
                    P_sb[:, 2 * mg:2 * mg + 2, ts], pp[:], EXP,
                    bias=0.0, scale=scale_attn)

            def scores_denom(nq):
                # deferred denominator: accumulated after the exps so the
                # scores matmuls themselves never pace to the ACT chain;
                # the recip is still a full step ahead of its consumer
                ts = slice(nq * FQ, (nq + 1) * FQ)
                psd = stat_tile(FQ)
                for t2 in range(MK // 2):
                    nc.tensor.matmul(
                        psd[:], ones2_f8[:, :, 0:1],
                        P_sb[:, 2 * t2:2 * t2 + 2, ts],
                        start=(t2 == 0), stop=(t2 == MK // 2 - 1),
                        perf_mode=DR,
                    )
                den = vecp.tile([1, FQ], F32, tag="vden")
                nc.vector.reciprocal_approx_fast(den[:], psd[:])
                nc.gpsimd.partition_broadcast(recip_bc[:, ts], den[0:1, :])

            def attn_chunk(nq, c):
                # attn^T chunk c = v^T P^T (x VS)
                ts = slice(nq * FQ, (nq + 1) * FQ)
                ps = psA.tile([P, FQ], F32, tag="main", name="psa")
                for t2 in range(MK // 2):
                    nc.tensor.matmul(
                        ps[:],
                        v_sb[:, 2 * t2:2 * t2 + 2, c * P:(c + 1) * P],
                        P_sb[:, 2 * t2:2 * t2 + 2, ts],
                        start=(t2 == 0),
                        stop=(t2 == MK // 2 - 1),
                        perf_mode=DR,
                    )
                nc.scalar.activation(attn8_sb[:, c, ts], ps[:], IDENT,
                                     scale=1.0 / (WS * VS))

            def emit_scores(nq):
                # standalone form (pre-loop / general path)
                ts = slice(nq * FQ, (nq + 1) * FQ)
                if trivial_bias:
                    for mg in range(MK // 2):
                        scores_group(nq, mg)
                    scores_denom(nq)
                else:
                    for mk in range(MK):
                        ps = psA.tile([P, FQ], F32, tag="main")
                        for t2 in range(C // 2):
                            nc.tensor.matmul(
                                ps[:],
                                kq_sb[:, 2 * t2:2 * t2 + 2, mk * P:(mk + 1) * P],
                                hf8_sb[:, 2 * t2:2 * t2 + 2, ts],
                                start=(t2 == 0),
                                stop=(t2 == C // 2 - 1),
                                perf_mode=DR,
                            )
                        nc.scalar.activation(
                            P_sb[:, mk, ts], ps[:], EXP,
                            bias=ck_sb[:, mk:mk + 1], scale=scale_attn)

            def emit_attn(nq):
                # attn^T = v^T P^T (x VS); denominator colsum after c==0
                ts = slice(nq * FQ, (nq + 1) * FQ)
                for c in range(C):
                    ps = psA.tile([P, FQ], F32, tag="main")
                    if trivial_bias:
                        for t2 in range(MK // 2):
                            nc.tensor.matmul(
                                ps[:],
                                v_sb[:, 2 * t2:2 * t2 + 2, c * P:(c + 1) * P],
                                P_sb[:, 2 * t2:2 * t2 + 2, ts],
                                start=(t2 == 0),
                                stop=(t2 == MK // 2 - 1),
                                perf_mode=DR,
                            )
                    else:
                        for mk in range(MK):
                            nc.tensor.matmul(
                                ps[:],
                                vbf_sb[:, mk, c * P:(c + 1) * P],
                                P_sb[:, mk, ts],
                                start=(mk == 0),
                                stop=(mk == MK - 1),
                            )
                    if c == 0 and not trivial_bias:
                        psd = stat_tile(FQ)
                        for mk in range(MK):
                            nc.tensor.matmul(
                                psd[:], ones_bf[:], P_sb[:, mk, ts],
                                start=(mk == 0),
                                stop=(mk == MK - 1),
                            )
                        den = vecp.tile([1, FQ], F32, tag="vden")
                        nc.vector.reciprocal_approx_fast(den[:], psd[:])
                        nc.gpsimd.partition_broadcast(
                            recip_bc[:, ts], den[0:1, :])
                    nc.scalar.activation(attn8_sb[:, c, ts], ps[:], IDENT,
                                         scale=1.0 / (WS * VS))

            def emit_ff1(nq):
                # ffn hidden: psum = attn@W1 (fp8 DR); softmax recip + relu
                # applied on the [64 x FQ] hidden (recip commutes through W1)
                ts = slice(nq * FQ, (nq + 1) * FQ)
                ps = psA.tile([HID, FQ], F32, tag="main")
                for t2 in range(C // 2):
                    nc.tensor.matmul(
                        ps[:], w1_sb[:, 2 * t2:2 * t2 + 2, :],
                        attn8_sb[:, 2 * t2:2 * t2 + 2, ts],
                        start=(t2 == 0), stop=(t2 == C // 2 - 1),
                        perf_mode=DR,
                    )
                nc.vector.tensor_mul(ps[:], ps[:], recip_bc[:HID, ts])
                nc.scalar.activation(
                    r_sb[:, ts], ps[:], RELU,
                    bias=b1_sb[:, 0:1], scale=scale_out,
                )

            def emit_ff2(nq, li, t0=None, tw=None, inline_norm=False,
                         out_last=False, cs=None):
                # ff2 + residual: t = W2^T r (+ b2) + h, add fused into the
                # PSUM->SBUF copy on DVE. t^2 is only materialized where the
                # LN stats can't be derived algebraically (layer 0 / general).
                # inline_norm: mu/alpha broadcasts are already out (stats ran
                # mid-iteration), so the normalize rides chunk-pair-wise right
                # behind the t-adds -- hf8 chunks 0-1 land early enough that
                # the next layer's scores never wait
                if t0 is None:
                    t0, tw = nq * FQ, FQ
                ts = slice(t0, t0 + tw)
                need_tsq = not (trivial_ln and trivial_bias)
                bs2 = (P, 2, tw)
                for c in cs if cs is not None else range(C):
                    ps = psA.tile([P, tw], F32, tag="main", name="psf")
                    nc.tensor.matmul(
                        ps[:], w2_sb[:, c * P:(c + 1) * P], r_sb[:, ts],
                        start=True, stop=trivial_bias,
                    )
                    if not trivial_bias:
                        nc.tensor.matmul(
                            ps[:], b2r_sb[0:1, c * P:(c + 1) * P],
                            ones_row[0:1, ts], start=False, stop=True,
                        )
                    nc.vector.tensor_tensor(t_sb[:, c, ts], ps[:],
                                            h_sb[:, c, ts], ADD)
                    if need_tsq:
                        nc.vector.tensor_mul(tsq_sb[:, c, ts], t_sb[:, c, ts],
                                             t_sb[:, c, ts])
                    if inline_norm and c % 2 == 1:
                        pc2 = slice(c - 1, c + 1)
                        nc.vector.tensor_tensor(
                            h_sb[:, pc2, ts], t_sb[:, pc2, ts],
                            mu1_bc[:, None, ts].to_broadcast(bs2), SUB,
                        )
                        if out_last:
                            nc.vector.tensor_tensor(
                                hout_sb[:, pc2, ts], h_sb[:, pc2, ts],
                                rstd1_bc[:, None, ts].to_broadcast(bs2),
                                MULT,
                            )
                            store_eng = [nc.sync, nc.scalar, nc.gpsimd,
                                         nc.sync]
                            for cc in (c - 1, c):
                                store_eng[cc].dma_start(
                                    out[cc, :, ts], hout_sb[:, cc, ts])
                        else:
                            nc.vector.tensor_tensor(
                                hf8_sb[:, pc2, ts], h_sb[:, pc2, ts],
                                rstd1_bc[:, None, ts].to_broadcast(bs2),
                                MULT,
                            )
                if inline_norm and not out_last and (cs is None or 3 in cs):
                    nc.vector.tensor_tensor(
                        h_sb[:, :, ts], h_sb[:, :, ts],
                        rstd1_bc[:, None, ts].to_broadcast((P, C, tw)), MULT,
                    )
                    if not trivial_ln:
                        if c == 1:
                            nc.vector.tensor_tensor(
                                stw_sb[:, 0, ts], t_sb[:, 0, ts],
                                t_sb[:, 1, ts], ADD)
                            nc.vector.tensor_tensor(
                                stw_sb[:, 1, ts], tsq_sb[:, 0, ts],
                                tsq_sb[:, 1, ts], ADD)
                        if c == 3:
                            nc.vector.tensor_tensor(
                                stw_sb[:, 0, ts], stw_sb[:, 0, ts],
                                t_sb[:, 2, ts], ADD)
                            nc.vector.tensor_tensor(
                                stw_sb[:, 0, ts], stw_sb[:, 0, ts],
                                t_sb[:, 3, ts], ADD)
                            nc.vector.tensor_tensor(
                                stw_sb[:, 1, ts], stw_sb[:, 1, ts],
                                tsq_sb[:, 2, ts], ADD)
                            nc.vector.tensor_tensor(
                                stw_sb[:, 1, ts], stw_sb[:, 1, ts],
                                tsq_sb[:, 3, ts], ADD)

            def layer_norm(src, dst, g, be, mu_bc, rstd_bc, nq, out_last=False,
                           use_stw=False):
                """General LN over the feature axis for token chunk nq.
                use_stw: the ff loop prebuilt sum(t)/sum(t^2) into stw_sb
                (valid for LN1 only; LN2 recomputes from its input)."""
                ts = slice(nq * FQ, (nq + 1) * FQ)
                if not use_stw:
                    nc.vector.tensor_mul(tsq_sb[:, :, ts], src[:, :, ts],
                                         src[:, :, ts])
                    nc.vector.tensor_tensor(
                        stw_sb[:, 0:1, ts], src[:, 0:1, ts], src[:, 1:2, ts], ADD)
                    nc.vector.tensor_tensor(
                        stw_sb[:, 0:1, ts], stw_sb[:, 0:1, ts], src[:, 2:3, ts], ADD)
                    nc.vector.tensor_tensor(
                        stw_sb[:, 0:1, ts], stw_sb[:, 0:1, ts], src[:, 3:4, ts], ADD)
                    nc.vector.tensor_tensor(
                        stw_sb[:, 1:2, ts], tsq_sb[:, 0:1, ts], tsq_sb[:, 1:2, ts], ADD)
                    nc.vector.tensor_tensor(
                        stw_sb[:, 1:2, ts], stw_sb[:, 1:2, ts], tsq_sb[:, 2:3, ts], ADD)
                    nc.vector.tensor_tensor(
                        stw_sb[:, 1:2, ts], stw_sb[:, 1:2, ts], tsq_sb[:, 3:4, ts], ADD)
                ps1 = stat_tile(FQ)
                nc.tensor.matmul(ps1[:], ones_bf[:], stw_sb[:, 0, ts],
                                 start=True, stop=True)
                ps2 = stat_tile(FQ)
                nc.tensor.matmul(ps2[:], ones_bf[:], stw_sb[:, 1, ts],
                                 start=True, stop=True)
                mu = vecp.tile([1, FQ], BF16, tag="v1")
                ev = vecp.tile([1, FQ], F32, tag="v2")
                msq = vecp.tile([1, FQ], F32, tag="v3")
                rstd = vecp.tile([1, FQ], BF16, tag="v4")
                nc.vector.tensor_scalar_mul(mu[:], ps1[:], 1.0 / D)
                nc.vector.tensor_scalar_mul(ev[:], ps2[:], 1.0 / D)
                nc.vector.tensor_mul(msq[:], mu[:], mu[:])
                nc.vector.tensor_tensor(ev[:], ev[:], msq[:], SUB)
                nc.scalar.activation(ev[:], ev[:], LN_, bias=eps_sb[:])
                nc.scalar.activation(rstd[:], ev[:], EXP, scale=-0.5)
                nc.gpsimd.partition_broadcast(mu_bc[:, ts], mu[0:1, :])
                nc.gpsimd.partition_broadcast(rstd_bc[:, ts], rstd[0:1, :])
                bshape = (P, C, FQ)
                nc.vector.tensor_tensor(
                    dst[:, :, ts], src[:, :, ts],
                    mu_bc[:, None, ts].to_broadcast(bshape), SUB,
                )
                nc.vector.tensor_tensor(
                    dst[:, :, ts], dst[:, :, ts],
                    rstd_bc[:, None, ts].to_broadcast(bshape), MULT,
                )
                dd = hout_sb if out_last else dst
                for c in range(C):
                    nc.vector.tensor_scalar(
                        dd[:, c, ts], dst[:, c, ts],
                        g[:, c:c + 1], be[:, c:c + 1], MULT, ADD,
                    )
                    if out_last:
                        nc.sync.dma_start(out[c, :, ts], hout_sb[:, c, ts])
                if not out_last and dst is not t_sb:
                    nc.vector.tensor_copy(hf8_sb[:, :, ts], dst[:, :, ts])

            def fused_ln_alpha(ps1, ps2, tw, ts, extra_var=0.0):
                """Shared mu/alpha tail: mu = ps1/D broadcast early; v1 =
                ps2/D - mu^2 + extra_var; alpha = rsqrt(v1(1+eps)+eps^2)
                via ln/exp. Broadcasts land in mu1_bc/rstd1_bc[:, ts]."""
                mu = vecp.tile([1, tw], BF16, tag="v1")
                ev = vecp.tile([1, tw], F32, tag="v2")
                msq = vecp.tile([1, tw], F32, tag="v3")
                alpha = vecp.tile([1, tw], BF16, tag="v6")
                nc.vector.tensor_scalar_mul(mu[:], ps1[:], 1.0 / D)
                nc.gpsimd.partition_broadcast(mu1_bc[:, ts], mu[0:1, :])
                # mu^2 on ACT (Square is in the restricted table) so the
                # DVE only carries one small op on this path
                nc.scalar.activation(msq[:], ps1[:],
                                     mybir.ActivationFunctionType.Square,
                                     scale=1.0 / D)
                nc.vector.scalar_tensor_tensor(
                    ev[:], ps2[:], 1.0 / D, msq[:], MULT, SUB)
                # r1*r2 = rsqrt((v1+eps)*(v2+eps)) with v2=v1/(v1+eps)
                #       = rsqrt(v1*(1+eps) + eps^2); extra_var folds into
                #       the ln bias: bias = extra*(1+eps) + eps^2
                bias = eps2_sb if extra_var == 0.0 else onep_sb
                nc.scalar.activation(ev[:], ev[:], LN_,
                                     bias=bias[:], scale=1.0 + EPS)
                nc.scalar.activation(alpha[:], ev[:], EXP, scale=-0.5)
                nc.gpsimd.partition_broadcast(rstd1_bc[:, ts], alpha[0:1, :])

            def fused_ln_stats_l0(t0, tw):
                """Layer-0 stats from t / t^2 directly (h = x there)."""
                ts = slice(t0, t0 + tw)
                ps1 = stat_tile(tw)
                for c in range(C):
                    nc.tensor.matmul(ps1[:], ones_bf[:], t_sb[:, c, ts],
                                     start=(c == 0), stop=(c == C - 1))
                ps2 = stat_tile(tw)
                for c in range(C):
                    nc.tensor.matmul(ps2[:], ones_bf[:], tsq_sb[:, c, ts],
                                     start=(c == 0), stop=(c == C - 1))
                fused_ln_alpha(ps1, ps2, tw, ts)

            def fused_ln_stats_a(nq, li):
                """Stats for layers > 0, derived from (r, hf8) only -- no
                dependence on t, so they run interleaved into the NEXT
                step's scores stream. colsum(t) = w2c @ r (LN output h is
                zero-mean); colsum(t^2) = r^T G r + 2 r^T (W2 h) + D (h is
                unit-var): G = W2 W2^T precomputed, W2 h is fp8-DR.
                Part a: mu (+broadcast) and the e = r*(Gr + 2W2h) product."""
                ts = slice(nq * FQ, (nq + 1) * FQ)
                ps1 = stat_tile(FQ)
                nc.tensor.matmul(ps1[:], w2c_sb[:], r_sb[:, ts],
                                 start=True, stop=(li > 0))
                if li == 0:
                    # h = x at layer 0: colsum(h) is the precomputed xsum
                    nc.tensor.matmul(ps1[:], ones_bf[0:1, :],
                                     xsum_sb[0:1, ts],
                                     start=False, stop=True)
                # one accumulation group: psE = 32*(G r + 2 W2 h) -- the
                # host ships g64 = G*32 (bf16) and w2t = W2^T*64 (fp8)
                psE = psA.tile([HID, FQ], F32, tag="main", name="psE")
                nc.tensor.matmul(psE[:], g64_sb[:], r_sb[:, ts],
                                 start=True, stop=False)
                for t2 in range(C // 2):
                    nc.tensor.matmul(
                        psE[:], w2t_sb[:, 2 * t2:2 * t2 + 2, :],
                        hf8_sb[:, 2 * t2:2 * t2 + 2, ts],
                        start=False, stop=(t2 == C // 2 - 1),
                        perf_mode=DR,
                    )
                mu = vecp.tile([1, FQ], BF16, tag="v1")
                msq = vecp.tile([1, FQ], F32, tag="v3")
                nc.vector.tensor_scalar_mul(mu[:], ps1[:], 1.0 / D)
                nc.gpsimd.partition_broadcast(mu1_bc[:, ts], mu[0:1, :])
                nc.scalar.activation(msq[:], ps1[:],
                                     mybir.ActivationFunctionType.Square,
                                     scale=1.0 / D)
                nc.vector.scalar_tensor_tensor(
                    e_sb[:, ts], psE[:], 1.0 / (WS / 2.0), r_sb[:, ts],
                    MULT, MULT)
                return msq

            def fused_ln_stats_b(nq, li, msq):
                """Part b: colsum(e) -> alpha chain -> rstd broadcast.
                Layer 0 accumulates the exact precomputed sum(x^2) instead
                of relying on the unit-variance identity."""
                ts = slice(nq * FQ, (nq + 1) * FQ)
                ps2 = stat_tile(FQ)
                nc.tensor.matmul(ps2[:], ones_bf[:HID, :], e_sb[:, ts],
                                 start=True, stop=(li > 0))
                if li == 0:
                    nc.tensor.matmul(ps2[:], ones_bf[0:1, :],
                                     xsqs_sb[0:1, ts],
                                     start=False, stop=True)
                ev = vecp.tile([1, FQ], F32, tag="v2")
                alpha = vecp.tile([1, FQ], BF16, tag="v6")
                nc.vector.scalar_tensor_tensor(
                    ev[:], ps2[:], 1.0 / D, msq[:], MULT, SUB)
                nc.scalar.activation(ev[:], ev[:], LN_,
                                     bias=eps2_sb[:] if li == 0 else onep_sb[:],
                                     scale=1.0 + EPS)
                nc.scalar.activation(alpha[:], ev[:], EXP, scale=-0.5)
                nc.gpsimd.partition_broadcast(rstd1_bc[:, ts], alpha[0:1, :])

            def fused_ln_norm(t0, tw, out_last=False):
                """Apply h = (t - mu)*alpha using the precomputed
                broadcasts. hf8 (chunks 0-1 first) gates the next layer's
                scores, so it is written before the bf16 h."""
                ts = slice(t0, t0 + tw)
                bshape = (P, C, tw)
                bs2 = (P, 2, tw)
                nc.vector.tensor_tensor(
                    h_sb[:, :, ts], t_sb[:, :, ts],
                    mu1_bc[:, None, ts].to_broadcast(bshape), SUB,
                )
                if out_last:
                    nc.vector.tensor_tensor(
                        hout_sb[:, :, ts], h_sb[:, :, ts],
                        rstd1_bc[:, None, ts].to_broadcast(bshape), MULT,
                    )
                    store_eng = [nc.sync, nc.scalar, nc.gpsimd, nc.sync]
                    for c in range(C):
                        store_eng[c].dma_start(out[c, :, ts],
                                               hout_sb[:, c, ts])
                else:
                    # hf8 chunk-pair 0-1 first: the next layer's first
                    # scores matmul needs only those chunks
                    for h2 in range(2):
                        nc.vector.tensor_tensor(
                            hf8_sb[:, 2 * h2:2 * h2 + 2, ts],
                            h_sb[:, 2 * h2:2 * h2 + 2, ts],
                            rstd1_bc[:, None, ts].to_broadcast(bs2), MULT,
                        )
                    nc.vector.tensor_tensor(
                        h_sb[:, :, ts], h_sb[:, :, ts],
                        rstd1_bc[:, None, ts].to_broadcast(bshape), MULT,
                    )

            # ---- transformer layers: flat chunk-step pipeline ----
            steps = [(li, nq) for li in range(L) for nq in range(NQ)]
            pending_ln = []

            fast = trivial_bias and trivial_ln
            emit_scores(steps[0][1])
            # per-token sum(x), sum(x^2) for the layer-0 LN stats; sits
            # behind the first scores block so the late h DMA is covered
            if fast:
                for nqq in range(NQ):
                    tsx = slice(nqq * FQ, (nqq + 1) * FQ)
                    nc.vector.tensor_mul(tsq_sb[:, :, tsx], h_sb[:, :, tsx],
                                         h_sb[:, :, tsx])
                    psx = stat_tile(FQ)
                    for c in range(C):
                        nc.tensor.matmul(psx[:], ones_bf[:], h_sb[:, c, tsx],
                                         start=(c == 0), stop=(c == C - 1))
                    nc.vector.tensor_copy(xsum_sb[0:1, tsx], psx[:])
                    psx2 = stat_tile(FQ)
                    for c in range(C):
                        nc.tensor.matmul(psx2[:], ones_bf[:],
                                         tsq_sb[:, c, tsx],
                                         start=(c == 0), stop=(c == C - 1))
                    nc.vector.tensor_copy(xsqs_sb[0:1, tsx], psx2[:])
            for i, (li, nq) in enumerate(steps):
                last = li == L - 1
                # previous step's pending work flushes here: the layer-0 LN
                # (or general-path LN) DVE chain hides under attn+scores
                while pending_ln:
                    pending_ln.pop(0)()

                if fast:
                    # merged steady-state: next step's scores groups
                    # interleave with this step's attn chunks so the exp
                    # chain starts ~4us earlier and attn matmuls fill its
                    # wait slots; LN stats (f(r, hf8) only) and ff1 slot
                    # into the scores tail
                    final = i == len(steps) - 1
                    if not final:
                        for u in range(4):
                            attn_chunk(nq, u)
                        emit_ff1(nq)
                        nxt = steps[i + 1][1]
                        msq = None
                        for g in range(MK // 2):
                            scores_group(nxt, g)
                            if g == 2:
                                msq = fused_ln_stats_a(nq, li)
                            elif g == 5:
                                fused_ln_stats_b(nq, li, msq)
                        # ff2 first half, then the deferred denominator
                        # (its partials wait on the exp tail), then the
                        # second half -- whose residual adds have had time
                        # to land by then
                        emit_ff2(nq, li, inline_norm=True, out_last=last,
                                 cs=(0, 1))
                        scores_denom(nxt)
                        emit_ff2(nq, li, inline_norm=True, out_last=last,
                                 cs=(2, 3))
                    else:
                        for u in range(4):
                            attn_chunk(nq, u)
                        emit_ff1(nq)
                        msq = fused_ln_stats_a(nq, li)
                        fused_ln_stats_b(nq, li, msq)
                        # final step: ff2 + normalize + store in
                        # half-windows so the tail drains incrementally
                        sw = FQ // 2
                        for j in range(2):
                            emit_ff2(nq, li, nq * FQ + j * sw, sw,
                                     inline_norm=True, out_last=True)
                    continue

                emit_attn(nq)
                emit_ff1(nq)
                if i + 1 < len(steps):
                    emit_scores(steps[i + 1][1])
                emit_ff2(nq, li)

                def _ln(nq=nq, last=last, li=li):
                    if trivial_ln:
                        fused_ln_stats_l0(nq * FQ, FQ)
                        fused_ln_norm(nq * FQ, FQ, out_last=last)
                    else:
                        layer_norm(t_sb, t_sb, g1_sb, be1_sb,
                                   mu1_bc, rstd1_bc, nq, use_stw=True)
                        layer_norm(t_sb, h_sb, g2_sb, be2_sb,
                                   mu2_bc, rstd2_bc, nq, out_last=last)
                pending_ln.append(_ln)
            while pending_ln:
                pending_ln.pop(0)()
    nc.compile()
    return nc


_NC_CACHE = {}


def _get_nc(trivial_ln, trivial_bias=False):
    key = ("nc", trivial_ln, trivial_bias)
    if key not in _NC_CACHE:
        _NC_CACHE[key] = build(trivial_ln=trivial_ln, trivial_bias=trivial_bias)
    return _NC_CACHE[key]


def _shard_inputs(x, Wq, bq, Wk, bk_, Wv, bv_, W1, b1, W2, b2, ln1_g, ln1_b, ln2_g, ln2_b):
    """Full inputs -> list of 8 per-core in_maps."""
    bf = ml_dtypes.bfloat16
    f8 = ml_dtypes.float8_e4m3
    C = D // P
    SQ = S // 2
    shared = {
        "wq": np.ascontiguousarray(Wq.T * WS).reshape(C, P, D).astype(f8),
        "wk": np.ascontiguousarray(Wk * WS).reshape(C, P, D).astype(f8),
        "wv": np.ascontiguousarray(Wv * WS).reshape(C, P, D).astype(f8),
        "w1": np.ascontiguousarray(W1 * WS).reshape(C, P, HID).astype(f8),
        "w2": np.ascontiguousarray(W2).astype(bf),
        "w2c": np.ascontiguousarray(W2.sum(axis=1).reshape(HID, 1)).astype(bf),
        "w2t": np.ascontiguousarray(W2.T * WS).reshape(C, P, HID).astype(f8),
        "g64": np.ascontiguousarray((W2 @ W2.T) * (WS / 2.0)).astype(bf),
        "bqc": np.ascontiguousarray(bq.reshape(C, P, 1)).astype(f8),
        "bk": np.ascontiguousarray(bk_.reshape(C, P).T).astype(np.float32),
        "bv": np.ascontiguousarray(bv_.reshape(1, D) * VS).astype(np.float32),
        "b1d": np.ascontiguousarray(b1.reshape(HID, 1)).astype(np.float32),
        "b2r": np.ascontiguousarray(b2.reshape(1, D)).astype(bf),
        "g1d": np.ascontiguousarray(ln1_g.reshape(C, P).T).astype(np.float32),
        "be1d": np.ascontiguousarray(ln1_b.reshape(C, P).T).astype(np.float32),
        "g2d": np.ascontiguousarray(ln2_g.reshape(C, P).T).astype(np.float32),
        "be2d": np.ascontiguousarray(ln2_b.reshape(C, P).T).astype(np.float32),
    }
    in_maps = []
    for core in range(8):
        b, j = core // 2, core % 2
        xT = np.ascontiguousarray(x[b].T)  # [D, S]
        xT8 = xT.astype(f8)
        m = dict(shared)
        m["xt"] = xT8.reshape(C, P, S)
        m["xq"] = np.ascontiguousarray(
            xT[:, j * SQ:(j + 1) * SQ].reshape(C, P, SQ)
        ).astype(bf)
        m["xq8"] = np.ascontiguousarray(
            xT8.reshape(C, P, S)[:, :, j * SQ:(j + 1) * SQ])
        in_maps.append(m)
    return in_maps


def _gather_output(results):
    SQ = S // 2
    out = np.empty((B, S, D), np.float32)
    for core, res in enumerate(results):
        b, j = core // 2, core % 2
        # res["out"]: [C, P, SQ] = h^T chunks -> h slice [SQ, D]
        out[b, j * SQ:(j + 1) * SQ, :] = (
            res["out"].astype(np.float32).reshape(D, SQ).T)
    return out


def _ln_trivial(inputs):
    return bool(
        np.all(inputs["ln1_g"] == 1.0) and np.all(inputs["ln1_b"] == 0.0)
        and np.all(inputs["ln2_g"] == 1.0) and np.all(inputs["ln2_b"] == 0.0)
    )


def _bias_trivial(inputs):
    return bool(all(np.all(inputs[k] == 0.0) for k in ("bq", "b2", "bv")))


def kernel(**inputs):
    nc = _get_nc(trivial_ln=_ln_trivial(inputs), trivial_bias=_bias_trivial(inputs))
    in_maps = _shard_inputs(
        inputs["x"], inputs["Wq"], inputs["bq"], inputs["Wk"], inputs["bk"],
        inputs["Wv"], inputs["bv"], inputs["W1"], inputs["b1"], inputs["W2"],
        inputs["b2"], inputs["ln1_g"], inputs["ln1_b"], inputs["ln2_g"],
        inputs["ln2_b"],
    )
    res = run_bass_kernel_spmd(nc, in_maps, core_ids=list(range(8)))
    return _gather_output(res.results)


# revision 41
# speedup vs baseline: 1.0010x; 1.0010x over previous
"""Blockwise-parallel transformer attention on 8 TRN2 NeuronCores.

Reference computation (per batch b):
    k = x@Wk + bk ; v = x@Wv + bv            (from ORIGINAL x, layer-invariant)
    h = x
    6x (shared weights):
        q = h@Wq + bq
        P = softmax(q k^T / 8)
        attn = (P @ v) / sqrt(512)
        ff = relu(attn@W1 + b1)@W2 + b2
        h = LN2(LN1(h + ff))

Sharding: 8 cores = 4 batches x 2 query-halves. Each core computes full
k/v for its batch (once), then processes its 1024-query slice through all
6 layers with zero cross-core traffic.

On-chip layout is fully transposed (feature dim on partitions, tokens on
the free axis); the host feeds x^T so the device never transposes.

Key algebraic move: scores = (h Wq + bq) k^T = h (k Wq^T)^T + bq k^T.
k is layer-invariant, so k' = k Wq^T is computed ONCE at setup and the
per-layer q projection disappears entirely; bq folds into the exp bias
via ck = (k@bq)/8. The residual add rides in the PSUM->SBUF copy
(DVE tensor_tensor) instead of an identity matmul.

fp8 numerics: fp8 weights are host-prescaled x64 (values ~0.02 would
otherwise sit in the fp8e4m3 subnormal range) and rescaled by 1/64 in
the PSUM->SBUF copies; v is stored x16 for the same reason. ff1 runs
fp8 DoubleRow on attn/1024 with W1*64.

Trivial-path (g=1,b=0,zero biases) specializations: LN2(LN1(t)) fuses
to (t-mu1)*alpha; softmax exps merge pairwise over 2-bank PSUM tiles
(ACT cost is per-column, so this nearly halves the exp chain); the
softmax-denominator partial sums ride the exp-wait slots of the scores
stream, one step ahead of their consumer; the LN statistics are derived
algebraically from (r, hf8) alone -- colsum(t) = w2c@r since the LN
output is zero-mean, colsum(t^2) = r^T(W2 W2^T)r + 2r^T(W2 h) + D since
it is unit-variance (layer 0 uses precomputed sum(x), sum(x^2)) -- so
they run interleaved in the scores stream two steps ahead of the
normalize, which itself rides chunk-pair-wise behind the ff2 residual
adds. Per-iteration PE order: attn(s), ff1(s), scores(s+1) x8 groups
(with denominator partials + LN stats of s in the gaps), ff2(s) with
inline normalize. The final step emits ff2+normalize+store in
half-windows so the tail drains incrementally.
"""

import sys

if "/opt/trn_rl_repo" not in sys.path:
    sys.path.insert(0, "/opt/trn_rl_repo")

import numpy as np
import ml_dtypes

import concourse.bass as bass
import concourse.mybir as mybir
import concourse.tile as tile
from concourse import bacc
import concourse.hw_specs as _hw_specs


def _restrict_act_tables():
    """All activation functions this kernel uses (exp, ln, relu, copy)
    live in the natural_log_exp_and_others table set. Left to its own
    devices the table-load pass alternates between exp_and_others and the
    ln set (~49 reloads x 1.5us of ACT time per run); restricting the
    offered sets collapses that to a single load. Dict order is preserved
    so act_func_set_id stays aligned with act_info.json."""
    if getattr(_hw_specs, "_act_tables_restricted", False):
        return
    orig = _hw_specs.get_activation_tables

    def restricted(arch):
        tables = orig(arch)
        return {
            name: (fns if name == "natural_log_exp_and_others" else set())
            for name, fns in tables.items()
        }

    _hw_specs.get_activation_tables = restricted
    bacc.get_activation_tables = restricted
    _hw_specs._act_tables_restricted = True


_restrict_act_tables()
from concourse.bass_utils import run_bass_kernel_spmd
from concourse.masks import make_identity

F32 = mybir.dt.float32
BF16 = mybir.dt.bfloat16
F8 = mybir.dt.float8e4
DR = mybir.MatmulPerfMode.DoubleRow
EXP = mybir.ActivationFunctionType.Exp
LN_ = mybir.ActivationFunctionType.Ln
RELU = mybir.ActivationFunctionType.Relu
IDENT = mybir.ActivationFunctionType.Identity
ADD = mybir.AluOpType.add
SUB = mybir.AluOpType.subtract
MULT = mybir.AluOpType.mult

B, S, D, HID, L = 4, 2048, 512, 64, 6
EPS = 1e-5
P = 128
WS = 64.0   # fp8 weight prescale (host x64, on-chip 1/64)
VS = 16.0   # v prescale


def build(S=S, SQ=S // 2, D=D, HID=HID, L=L, trivial_ln=False, trivial_bias=False):
    """Build + compile the per-core Bass program (same program on all 8 cores)."""
    C = D // P          # feature-dim 128-chunks (4)
    MK = S // P         # key-token 128-chunks (16)
    FK = min(512, S)    # key free-dim tile
    NK = S // FK
    FQ = min(512, SQ)   # query free-dim tile
    NQ = SQ // FQ
    scale_attn = 1.0 / float(np.sqrt(HID))
    scale_out = 1.0 / float(np.sqrt(D))

    nc = bacc.Bacc("TRN2", target_bir_lowering=False, debug=False)

    # ---- DRAM I/O (per core) ----
    xt = nc.dram_tensor("xt", (C, P, S), F8, kind="ExternalInput")
    xq = nc.dram_tensor("xq", (C, P, SQ), BF16, kind="ExternalInput")
    xq8 = nc.dram_tensor("xq8", (C, P, SQ), F8, kind="ExternalInput")
    wqT = nc.dram_tensor("wq", (C, P, D), F8, kind="ExternalInput")   # Wq^T x64
    wk = nc.dram_tensor("wk", (C, P, D), F8, kind="ExternalInput")    # Wk x64
    wv = nc.dram_tensor("wv", (C, P, D), F8, kind="ExternalInput")    # Wv x64
    w1 = nc.dram_tensor("w1", (C, P, HID), F8, kind="ExternalInput")  # W1 x64
    w2 = nc.dram_tensor("w2", (HID, D), BF16, kind="ExternalInput")
    w2c = nc.dram_tensor("w2c", (HID, 1), BF16, kind="ExternalInput")
    w2t = nc.dram_tensor("w2t", (C, P, HID), F8, kind="ExternalInput")  # W2^T x64
    g64 = nc.dram_tensor("g64", (HID, HID), BF16, kind="ExternalInput")  # W2 W2^T
    bqc = nc.dram_tensor("bqc", (C, P, 1), F8, kind="ExternalInput")
    bk = nc.dram_tensor("bk", (P, C), F32, kind="ExternalInput")
    bv = nc.dram_tensor("bv", (1, D), F32, kind="ExternalInput")      # bv x16
    b1d = nc.dram_tensor("b1d", (HID, 1), F32, kind="ExternalInput")
    b2r = nc.dram_tensor("b2r", (1, D), BF16, kind="ExternalInput")
    g1d = nc.dram_tensor("g1d", (P, C), F32, kind="ExternalInput")
    be1d = nc.dram_tensor("be1d", (P, C), F32, kind="ExternalInput")
    g2d = nc.dram_tensor("g2d", (P, C), F32, kind="ExternalInput")
    be2d = nc.dram_tensor("be2d", (P, C), F32, kind="ExternalInput")
    out = nc.dram_tensor("out", (C, P, SQ), BF16, kind="ExternalOutput")

    # PSUM bank budget (8 banks):
    #  trivial: psA x4 [P,FQ] (attn/ff) + psP x2 of 2 banks (score pairs,
    #           LN stats and softmax denominator share psP's rotation)
    #  general: psA x6 + psS x2 (baseline layout)
    nA = 4 if trivial_bias else 6
    with tile.TileContext(nc) as tc:
        with (
            tc.tile_pool(name="const", bufs=1) as cons,
            tc.tile_pool(name="big", bufs=1) as big,
            tc.tile_pool(name="vec", bufs=2) as vecp,
            tc.tile_pool(name="psA", bufs=nA, space="PSUM") as psA,
            tc.tile_pool(name="psS", bufs=2, space="PSUM") as psS,
            tc.tile_pool(name="psP", bufs=2, space="PSUM") as psP,
        ):
            def stat_tile(tw):
                if trivial_bias:
                    return psP.tile([1, tw], F32, tag="pp", name="pstat")
                return psS.tile([1, tw], F32, tag="stat", name="pstat")

            # ---- persistent SBUF ----
            wqT_sb = cons.tile([P, C, D], F8)
            wk_sb = cons.tile([P, C, D], F8)
            wv_sb = cons.tile([P, C, D], F8)
            w1_sb = cons.tile([P, C, HID], F8)
            w2_sb = cons.tile([HID, D], BF16)
            w2c_sb = cons.tile([HID, 1], BF16)
            w2t_sb = cons.tile([P, C, HID], F8)
            g64_sb = cons.tile([HID, HID], BF16)
            e_sb = cons.tile([HID, SQ], BF16)   # r*(Gr + 2 W2 h) for var stats
            xsum_sb = cons.tile([1, SQ], BF16)  # per-token sum(x) (layer 0)
            xsqs_sb = cons.tile([1, SQ], BF16)  # per-token sum(x^2) (layer 0)
            onep_sb = cons.tile([1, 1], F32)    # 1 + eps + eps^2
            bqc_sb = cons.tile([P, C], F8)
            bk_sb = cons.tile([P, C], F32)
            bv_sb = cons.tile([1, D], F32)
            bv_bc = cons.tile([P, D], F32)
            b1_sb = cons.tile([HID, 1], F32)
            b2r_sb = cons.tile([1, D], BF16)
            g1_sb = cons.tile([P, C], F32)
            be1_sb = cons.tile([P, C], F32)
            g2_sb = cons.tile([P, C], F32)
            be2_sb = cons.tile([P, C], F32)
            ones_bf = cons.tile([P, 1], BF16)
            ones2_f8 = cons.tile([P, 2, 16], F8)  # pair-dim stride must be 16B
            ones_row = cons.tile([1, SQ], BF16)
            eps_sb = cons.tile([1, 1], F32)
            eps2_sb = cons.tile([1, 1], F32)
            ident_sb = cons.tile([P, P], BF16)
            ck_sb = cons.tile([P, MK], F32)   # exp bias: (k @ bq)/8 per key token

            k_sb = cons.tile([P, C, S], F8)       # k^T (fp8)
            kq_sb = cons.tile([P, C, S], F8)      # k'^T = Wq k^T (fp8)
            v_sb = cons.tile([P, MK, D], F8)      # v natural x16 (fp8)
            vbf_sb = None
            if not trivial_bias:
                vbf_sb = cons.tile([P, MK, D], BF16)
            h_sb = cons.tile([P, C, SQ], BF16)    # h^T (residual stream)
            hf8_sb = cons.tile([P, C, SQ], F8)    # h^T in fp8 for the scores matmul
            attn8_sb = cons.tile([P, C, SQ], F8)  # attn^T / (WS*VS) in fp8
            r_sb = cons.tile([HID, SQ], BF16)     # relu(ffn hidden)
            t_sb = cons.tile([P, C, SQ], BF16)    # residual pre-LN
            tsq_sb = cons.tile([P, C, SQ], BF16)
            stw_sb = None
            if not trivial_ln:
                stw_sb = cons.tile([P, 2, SQ], BF16)  # [sum(t), sum(t^2)]
            hout_sb = cons.tile([P, C, SQ], BF16)  # final-layer output
            recip_bc = cons.tile([P, SQ], F32)
            mu1_bc = cons.tile([P, SQ], BF16)
            rstd1_bc = cons.tile([P, SQ], BF16)
            mu2_bc = cons.tile([P, SQ], BF16)
            rstd2_bc = cons.tile([P, SQ], BF16)

            # fp8 P is safe only with zero biases (logits stay in ~[-3,3]);
            # the general path keeps bf16 P and standard matmuls
            P_dt = F8 if trivial_bias else BF16
            P_sb = cons.tile([P, MK, SQ], P_dt)  # exp(scores^T)

            xt_sb = big.tile([P, C, S], F8, tag="bigshare")

            # ---- constants first so the PE warmup isn't gated on queue
            # drain, then inputs spread across engine queues (each dma_start
            # costs ~600ns of issue time on its queue) ----
            nc.vector.memset(ones_bf[:], 1.0)
            nc.vector.memset(ones2_f8[:], 1.0)
            nc.vector.memset(eps_sb[:], EPS)
            nc.vector.memset(eps2_sb[:], EPS * EPS)
            nc.vector.memset(onep_sb[:], 1.0 + EPS + EPS * EPS)
            if not trivial_bias:
                nc.vector.memset(ones_row[:], 1.0)
            make_identity(nc, ident_sb[:])

            for c in range(C):
                nc.sync.dma_start(wk_sb[:, c, :], wk[c, :, :])
            for c in range(C):
                nc.sync.dma_start(xt_sb[:, c, 0:S // 2], xt[c, :, 0:S // 2])
            for c in range(C):
                nc.sync.dma_start(xt_sb[:, c, S // 2:S], xt[c, :, S // 2:S])
            nc.gpsimd.dma_start(bk_sb[:], bk[:, :])
            for c in range(C):
                nc.gpsimd.dma_start(wqT_sb[:, c, :], wqT[c, :, :])
            for c in range(C):
                nc.gpsimd.dma_start(wv_sb[:, c, :], wv[c, :, :])
            nc.gpsimd.dma_start(w1_sb[:], w1[:, :, :].rearrange("c p d -> p c d"))
            nc.gpsimd.dma_start(w2_sb[:], w2[:, :])
            nc.gpsimd.dma_start(w2c_sb[:], w2c[:, :])
            nc.gpsimd.dma_start(w2t_sb[:], w2t[:, :, :].rearrange("c p d -> p c d"))
            nc.gpsimd.dma_start(g64_sb[:], g64[:, :])
            for c in range(C):
                nc.scalar.dma_start(hf8_sb[:, c, :], xq8[c, :, :])
            nc.gpsimd.dma_start(b1_sb[:], b1d[:, :])
            if not trivial_bias:
                nc.gpsimd.dma_start(bqc_sb[:], bqc[:, :, 0].rearrange("c p -> p c"))
                nc.gpsimd.dma_start(bv_sb[:], bv[:, :])
                nc.gpsimd.dma_start(b2r_sb[:], b2r[:, :])
            if not trivial_ln:
                nc.gpsimd.dma_start(g1_sb[:], g1d[:, :])
                nc.gpsimd.dma_start(be1_sb[:], be1d[:, :])
                nc.gpsimd.dma_start(g2_sb[:], g2d[:, :])
                nc.gpsimd.dma_start(be2_sb[:], be2d[:, :])
            if not trivial_bias:
                nc.gpsimd.partition_broadcast(bv_bc[:], bv_sb[0:1, :])

            # HAM warmup: keep the PE busy while the input DMAs land so the
            # first real matmuls run at full clock
            wu = psA.tile([P, P], F32, tag="main")
            for _ in range(80):
                nc.tensor.matmul(wu[:], ident_sb[:], ident_sb[:],
                                 start=True, stop=True)

            # ---- k^T = (Wk^T x^T)/WS + bk ;  k'^T = (Wq k^T)/WS ----
            def k_tile(nk):
                for c in range(C):
                    ps = psA.tile([P, FK], F32, tag="main")
                    for t2 in range(C // 2):
                        nc.tensor.matmul(
                            ps[:],
                            wk_sb[:, 2 * t2:2 * t2 + 2, c * P:(c + 1) * P],
                            xt_sb[:, 2 * t2:2 * t2 + 2, nk * FK:(nk + 1) * FK],
                            start=(t2 == 0),
                            stop=(t2 == C // 2 - 1),
                            perf_mode=DR,
                        )
                    nc.scalar.activation(
                        k_sb[:, c, nk * FK:(nk + 1) * FK], ps[:], IDENT,
                        bias=bk_sb[:, c:c + 1], scale=1.0 / WS,
                    )

            def kq_tile(nk):
                for c in range(C):
                    ps = psA.tile([P, FK], F32, tag="main")
                    for t2 in range(C // 2):
                        nc.tensor.matmul(
                            ps[:],
                            wqT_sb[:, 2 * t2:2 * t2 + 2, c * P:(c + 1) * P],
                            k_sb[:, 2 * t2:2 * t2 + 2, nk * FK:(nk + 1) * FK],
                            start=(t2 == 0),
                            stop=(t2 == C // 2 - 1),
                            perf_mode=DR,
                        )
                    if c % 2 == 0:
                        nc.scalar.activation(
                            kq_sb[:, c, nk * FK:(nk + 1) * FK], ps[:], IDENT,
                            scale=1.0 / WS)
                    else:
                        nc.vector.tensor_scalar_mul(
                            kq_sb[:, c, nk * FK:(nk + 1) * FK], ps[:], 1.0 / WS)

            k_tile(0)
            k_tile(1)
            kq_tile(0)
            k_tile(2)
            kq_tile(1)
            k_tile(3)
            kq_tile(2)
            kq_tile(3)

            # h (bf16) is needed first by the layer-0 x-stats (~20us in);
            # its DMA issues late so it doesn't compete with the k-path
            for c in range(C):
                nc.gpsimd.dma_start(h_sb[:, c, :], xq[c, :, :])



            # ---- ck = (k @ bq) * scale_attn  (exp bias; layer-invariant) ----
            for mk in range(MK) if not trivial_bias else []:
                ps = psS.tile([P, 1], F32, tag="stat")
                for c in range(C):
                    nc.tensor.matmul(
                        ps[:],
                        k_sb[:, c, mk * P:(mk + 1) * P],
                        bqc_sb[:, c:c + 1],
                        start=(c == 0),
                        stop=(c == C - 1),
                    )
                nc.vector.tensor_scalar_mul(ck_sb[:, mk:mk + 1], ps[:], scale_attn)

            # ---- v = (x@Wv)*VS/WS + bv*VS ----
            for mk in range(MK):
                ps = psA.tile([P, D], F32, tag="main")
                for t2 in range(C // 2):
                    nc.tensor.matmul(
                        ps[:],
                        xt_sb[:, 2 * t2:2 * t2 + 2, mk * P:(mk + 1) * P],
                        wv_sb[:, 2 * t2:2 * t2 + 2, :],
                        start=(t2 == 0),
                        stop=(t2 == C // 2 - 1),
                        perf_mode=DR,
                    )
                if trivial_bias:
                    # bv == 0 in the trivial path: plain scaled copy
                    nc.vector.tensor_scalar_mul(v_sb[:, mk, :], ps[:], VS / WS)
                else:
                    nc.vector.scalar_tensor_tensor(
                        v_sb[:, mk, :], ps[:], VS / WS, bv_bc[:], MULT, ADD)
                    nc.vector.scalar_tensor_tensor(
                        vbf_sb[:, mk, :], ps[:], VS / WS, bv_bc[:], MULT, ADD)

            # ---- per-chunk pipeline pieces ----
            def scores_begin():
                # softmax-denominator accumulator; its partials ride the
                # exp-wait slots of the 2-deep psP rotation
                return psA.tile([1, FQ], F32, tag="main", name="psd")

            def scores_group(nq, mg, psd):
                # one pair of key chunks shares a 2-bank PSUM tile -> one
                # [128, 2*FQ] exp per pair (ACT cost is per-column)
                ts = slice(nq * FQ, (nq + 1) * FQ)
                pp = psP.tile([P, 2 * FQ], F32, tag="pp")
                for half in range(2):
                    mk = 2 * mg + half
                    for t2 in range(C // 2):
                        nc.tensor.matmul(
                            pp[:, half * FQ:(half + 1) * FQ],
                            kq_sb[:, 2 * t2:2 * t2 + 2,
                                  mk * P:(mk + 1) * P],
                            hf8_sb[:, 2 * t2:2 * t2 + 2, ts],
                            start=(t2 == 0),
                            stop=(t2 == C // 2 - 1),
                            perf_mode=DR,
                        )
                nc.scalar.activation(
                    P_sb[:, 2 * mg:2 * mg + 2, ts], pp[:], EXP,
                    bias=0.0, scale=scale_attn)
                nc.tensor.matmul(
                    psd[:], ones2_f8[:, :, 0:1],
                    P_sb[:, 2 * mg:2 * mg + 2, ts],
                    start=(mg == 0), stop=(mg == MK // 2 - 1),
                    perf_mode=DR,
                )

            def scores_end(nq, psd):
                ts = slice(nq * FQ, (nq + 1) * FQ)
                den = vecp.tile([1, FQ], F32, tag="vden")
                nc.vector.reciprocal_approx_fast(den[:], psd[:])
                nc.gpsimd.partition_broadcast(recip_bc[:, ts], den[0:1, :])

            def attn_chunk(nq, c):
                # attn^T chunk c = v^T P^T (x VS)
                ts = slice(nq * FQ, (nq + 1) * FQ)
                ps = psA.tile([P, FQ], F32, tag="main", name="psa")
                for t2 in range(MK // 2):
                    nc.tensor.matmul(
                        ps[:],
                        v_sb[:, 2 * t2:2 * t2 + 2, c * P:(c + 1) * P],
                        P_sb[:, 2 * t2:2 * t2 + 2, ts],
                        start=(t2 == 0),
                        stop=(t2 == MK // 2 - 1),
                        perf_mode=DR,
                    )
                nc.scalar.activation(attn8_sb[:, c, ts], ps[:], IDENT,
                                     scale=1.0 / (WS * VS))

            def emit_scores(nq):
                # standalone form (pre-loop / general path)
                ts = slice(nq * FQ, (nq + 1) * FQ)
                if trivial_bias:
                    psd = scores_begin()
                    for mg in range(MK // 2):
                        scores_group(nq, mg, psd)
                    scores_end(nq, psd)
                else:
                    for mk in range(MK):
                        ps = psA.tile([P, FQ], F32, tag="main")
                        for t2 in range(C // 2):
                            nc.tensor.matmul(
                                ps[:],
                                kq_sb[:, 2 * t2:2 * t2 + 2, mk * P:(mk + 1) * P],
                                hf8_sb[:, 2 * t2:2 * t2 + 2, ts],
                                start=(t2 == 0),
                                stop=(t2 == C // 2 - 1),
                                perf_mode=DR,
                            )
                        nc.scalar.activation(
                            P_sb[:, mk, ts], ps[:], EXP,
                            bias=ck_sb[:, mk:mk + 1], scale=scale_attn)

            def emit_attn(nq):
                # attn^T = v^T P^T (x VS); denominator colsum after c==0
                ts = slice(nq * FQ, (nq + 1) * FQ)
                for c in range(C):
                    ps = psA.tile([P, FQ], F32, tag="main")
                    if trivial_bias:
                        for t2 in range(MK // 2):
                            nc.tensor.matmul(
                                ps[:],
                                v_sb[:, 2 * t2:2 * t2 + 2, c * P:(c + 1) * P],
                                P_sb[:, 2 * t2:2 * t2 + 2, ts],
                                start=(t2 == 0),
                                stop=(t2 == MK // 2 - 1),
                                perf_mode=DR,
                            )
                    else:
                        for mk in range(MK):
                            nc.tensor.matmul(
                                ps[:],
                                vbf_sb[:, mk, c * P:(c + 1) * P],
                                P_sb[:, mk, ts],
                                start=(mk == 0),
                                stop=(mk == MK - 1),
                            )
                    if c == 0 and not trivial_bias:
                        psd = stat_tile(FQ)
                        for mk in range(MK):
                            nc.tensor.matmul(
                                psd[:], ones_bf[:], P_sb[:, mk, ts],
                                start=(mk == 0),
                                stop=(mk == MK - 1),
                            )
                        den = vecp.tile([1, FQ], F32, tag="vden")
                        nc.vector.reciprocal_approx_fast(den[:], psd[:])
                        nc.gpsimd.partition_broadcast(
                            recip_bc[:, ts], den[0:1, :])
                    nc.scalar.activation(attn8_sb[:, c, ts], ps[:], IDENT,
                                         scale=1.0 / (WS * VS))

            def emit_ff1(nq):
                # ffn hidden: psum = attn@W1 (fp8 DR); softmax recip + relu
                # applied on the [64 x FQ] hidden (recip commutes through W1)
                ts = slice(nq * FQ, (nq + 1) * FQ)
                ps = psA.tile([HID, FQ], F32, tag="main")
                for t2 in range(C // 2):
                    nc.tensor.matmul(
                        ps[:], w1_sb[:, 2 * t2:2 * t2 + 2, :],
                        attn8_sb[:, 2 * t2:2 * t2 + 2, ts],
                        start=(t2 == 0), stop=(t2 == C // 2 - 1),
                        perf_mode=DR,
                    )
                nc.vector.tensor_mul(ps[:], ps[:], recip_bc[:HID, ts])
                nc.scalar.activation(
                    r_sb[:, ts], ps[:], RELU,
                    bias=b1_sb[:, 0:1], scale=scale_out,
                )

            def emit_ff2(nq, li, t0=None, tw=None, inline_norm=False,
                         out_last=False):
                # ff2 + residual: t = W2^T r (+ b2) + h, add fused into the
                # PSUM->SBUF copy on DVE. t^2 is only materialized where the
                # LN stats can't be derived algebraically (layer 0 / general).
                # inline_norm: mu/alpha broadcasts are already out (stats ran
                # mid-iteration), so the normalize rides chunk-pair-wise right
                # behind the t-adds -- hf8 chunks 0-1 land early enough that
                # the next layer's scores never wait
                if t0 is None:
                    t0, tw = nq * FQ, FQ
                ts = slice(t0, t0 + tw)
                need_tsq = not (trivial_ln and trivial_bias)
                bs2 = (P, 2, tw)
                for c in range(C):
                    ps = psA.tile([P, tw], F32, tag="main", name="psf")
                    nc.tensor.matmul(
                        ps[:], w2_sb[:, c * P:(c + 1) * P], r_sb[:, ts],
                        start=True, stop=trivial_bias,
                    )
                    if not trivial_bias:
                        nc.tensor.matmul(
                            ps[:], b2r_sb[0:1, c * P:(c + 1) * P],
                            ones_row[0:1, ts], start=False, stop=True,
                        )
                    nc.vector.tensor_tensor(t_sb[:, c, ts], ps[:],
                                            h_sb[:, c, ts], ADD)
                    if need_tsq:
                        nc.vector.tensor_mul(tsq_sb[:, c, ts], t_sb[:, c, ts],
                                             t_sb[:, c, ts])
                    if inline_norm and c % 2 == 1:
                        pc2 = slice(c - 1, c + 1)
                        nc.vector.tensor_tensor(
                            h_sb[:, pc2, ts], t_sb[:, pc2, ts],
                            mu1_bc[:, None, ts].to_broadcast(bs2), SUB,
                        )
                        if out_last:
                            nc.vector.tensor_tensor(
                                hout_sb[:, pc2, ts], h_sb[:, pc2, ts],
                                rstd1_bc[:, None, ts].to_broadcast(bs2),
                                MULT,
                            )
                            store_eng = [nc.sync, nc.scalar, nc.gpsimd,
                                         nc.sync]
                            for cc in (c - 1, c):
                                store_eng[cc].dma_start(
                                    out[cc, :, ts], hout_sb[:, cc, ts])
                        else:
                            nc.vector.tensor_tensor(
                                hf8_sb[:, pc2, ts], h_sb[:, pc2, ts],
                                rstd1_bc[:, None, ts].to_broadcast(bs2),
                                MULT,
                            )
                if inline_norm and not out_last:
                    nc.vector.tensor_tensor(
                        h_sb[:, :, ts], h_sb[:, :, ts],
                        rstd1_bc[:, None, ts].to_broadcast((P, C, tw)), MULT,
                    )
                    if not trivial_ln:
                        if c == 1:
                            nc.vector.tensor_tensor(
                                stw_sb[:, 0, ts], t_sb[:, 0, ts],
                                t_sb[:, 1, ts], ADD)
                            nc.vector.tensor_tensor(
                                stw_sb[:, 1, ts], tsq_sb[:, 0, ts],
                                tsq_sb[:, 1, ts], ADD)
                        if c == 3:
                            nc.vector.tensor_tensor(
                                stw_sb[:, 0, ts], stw_sb[:, 0, ts],
                                t_sb[:, 2, ts], ADD)
                            nc.vector.tensor_tensor(
                                stw_sb[:, 0, ts], stw_sb[:, 0, ts],
                                t_sb[:, 3, ts], ADD)
                            nc.vector.tensor_tensor(
                                stw_sb[:, 1, ts], stw_sb[:, 1, ts],
                                tsq_sb[:, 2, ts], ADD)
                            nc.vector.tensor_tensor(
                                stw_sb[:, 1, ts], stw_sb[:, 1, ts],
                                tsq_sb[:, 3, ts], ADD)

            def layer_norm(src, dst, g, be, mu_bc, rstd_bc, nq, out_last=False,
                           use_stw=False):
                """General LN over the feature axis for token chunk nq.
                use_stw: the ff loop prebuilt sum(t)/sum(t^2) into stw_sb
                (valid for LN1 only; LN2 recomputes from its input)."""
                ts = slice(nq * FQ, (nq + 1) * FQ)
                if not use_stw:
                    nc.vector.tensor_mul(tsq_sb[:, :, ts], src[:, :, ts],
                                         src[:, :, ts])
                    nc.vector.tensor_tensor(
                        stw_sb[:, 0:1, ts], src[:, 0:1, ts], src[:, 1:2, ts], ADD)
                    nc.vector.tensor_tensor(
                        stw_sb[:, 0:1, ts], stw_sb[:, 0:1, ts], src[:, 2:3, ts], ADD)
                    nc.vector.tensor_tensor(
                        stw_sb[:, 0:1, ts], stw_sb[:, 0:1, ts], src[:, 3:4, ts], ADD)
                    nc.vector.tensor_tensor(
                        stw_sb[:, 1:2, ts], tsq_sb[:, 0:1, ts], tsq_sb[:, 1:2, ts], ADD)
                    nc.vector.tensor_tensor(
                        stw_sb[:, 1:2, ts], stw_sb[:, 1:2, ts], tsq_sb[:, 2:3, ts], ADD)
                    nc.vector.tensor_tensor(
                        stw_sb[:, 1:2, ts], stw_sb[:, 1:2, ts], tsq_sb[:, 3:4, ts], ADD)
                ps1 = stat_tile(FQ)
                nc.tensor.matmul(ps1[:], ones_bf[:], stw_sb[:, 0, ts],
                                 start=True, stop=True)
                ps2 = stat_tile(FQ)
                nc.tensor.matmul(ps2[:], ones_bf[:], stw_sb[:, 1, ts],
                                 start=True, stop=True)
                mu = vecp.tile([1, FQ], BF16, tag="v1")
                ev = vecp.tile([1, FQ], F32, tag="v2")
                msq = vecp.tile([1, FQ], F32, tag="v3")
                rstd = vecp.tile([1, FQ], BF16, tag="v4")
                nc.vector.tensor_scalar_mul(mu[:], ps1[:], 1.0 / D)
                nc.vector.tensor_scalar_mul(ev[:], ps2[:], 1.0 / D)
                nc.vector.tensor_mul(msq[:], mu[:], mu[:])
                nc.vector.tensor_tensor(ev[:], ev[:], msq[:], SUB)
                nc.scalar.activation(ev[:], ev[:], LN_, bias=eps_sb[:])
                nc.scalar.activation(rstd[:], ev[:], EXP, scale=-0.5)
                nc.gpsimd.partition_broadcast(mu_bc[:, ts], mu[0:1, :])
                nc.gpsimd.partition_broadcast(rstd_bc[:, ts], rstd[0:1, :])
                bshape = (P, C, FQ)
                nc.vector.tensor_tensor(
                    dst[:, :, ts], src[:, :, ts],
                    mu_bc[:, None, ts].to_broadcast(bshape), SUB,
                )
                nc.vector.tensor_tensor(
                    dst[:, :, ts], dst[:, :, ts],
                    rstd_bc[:, None, ts].to_broadcast(bshape), MULT,
                )
                dd = hout_sb if out_last else dst
                for c in range(C):
                    nc.vector.tensor_scalar(
                        dd[:, c, ts], dst[:, c, ts],
                        g[:, c:c + 1], be[:, c:c + 1], MULT, ADD,
                    )
                    if out_last:
                        nc.sync.dma_start(out[c, :, ts], hout_sb[:, c, ts])
                if not out_last and dst is not t_sb:
                    nc.vector.tensor_copy(hf8_sb[:, :, ts], dst[:, :, ts])

            def fused_ln_alpha(ps1, ps2, tw, ts, extra_var=0.0):
                """Shared mu/alpha tail: mu = ps1/D broadcast early; v1 =
                ps2/D - mu^2 + extra_var; alpha = rsqrt(v1(1+eps)+eps^2)
                via ln/exp. Broadcasts land in mu1_bc/rstd1_bc[:, ts]."""
                mu = vecp.tile([1, tw], BF16, tag="v1")
                ev = vecp.tile([1, tw], F32, tag="v2")
                msq = vecp.tile([1, tw], F32, tag="v3")
                alpha = vecp.tile([1, tw], BF16, tag="v6")
                nc.vector.tensor_scalar_mul(mu[:], ps1[:], 1.0 / D)
                nc.gpsimd.partition_broadcast(mu1_bc[:, ts], mu[0:1, :])
                # mu^2 on ACT (Square is in the restricted table) so the
                # DVE only carries one small op on this path
                nc.scalar.activation(msq[:], ps1[:],
                                     mybir.ActivationFunctionType.Square,
                                     scale=1.0 / D)
                nc.vector.scalar_tensor_tensor(
                    ev[:], ps2[:], 1.0 / D, msq[:], MULT, SUB)
                # r1*r2 = rsqrt((v1+eps)*(v2+eps)) with v2=v1/(v1+eps)
                #       = rsqrt(v1*(1+eps) + eps^2); extra_var folds into
                #       the ln bias: bias = extra*(1+eps) + eps^2
                bias = eps2_sb if extra_var == 0.0 else onep_sb
                nc.scalar.activation(ev[:], ev[:], LN_,
                                     bias=bias[:], scale=1.0 + EPS)
                nc.scalar.activation(alpha[:], ev[:], EXP, scale=-0.5)
                nc.gpsimd.partition_broadcast(rstd1_bc[:, ts], alpha[0:1, :])

            def fused_ln_stats_l0(t0, tw):
                """Layer-0 stats from t / t^2 directly (h = x there)."""
                ts = slice(t0, t0 + tw)
                ps1 = stat_tile(tw)
                for c in range(C):
                    nc.tensor.matmul(ps1[:], ones_bf[:], t_sb[:, c, ts],
                                     start=(c == 0), stop=(c == C - 1))
                ps2 = stat_tile(tw)
                for c in range(C):
                    nc.tensor.matmul(ps2[:], ones_bf[:], tsq_sb[:, c, ts],
                                     start=(c == 0), stop=(c == C - 1))
                fused_ln_alpha(ps1, ps2, tw, ts)

            def fused_ln_stats_a(nq, li):
                """Stats for layers > 0, derived from (r, hf8) only -- no
                dependence on t, so they run interleaved into the NEXT
                step's scores stream. colsum(t) = w2c @ r (LN output h is
                zero-mean); colsum(t^2) = r^T G r + 2 r^T (W2 h) + D (h is
                unit-var): G = W2 W2^T precomputed, W2 h is fp8-DR.
                Part a: mu (+broadcast) and the e = r*(Gr + 2W2h) product."""
                ts = slice(nq * FQ, (nq + 1) * FQ)
                ps1 = stat_tile(FQ)
                nc.tensor.matmul(ps1[:], w2c_sb[:], r_sb[:, ts],
                                 start=True, stop=(li > 0))
                if li == 0:
                    # h = x at layer 0: colsum(h) is the precomputed xsum
                    nc.tensor.matmul(ps1[:], ones_bf[0:1, :],
                                     xsum_sb[0:1, ts],
                                     start=False, stop=True)
                # one accumulation group: psE = 32*(G r + 2 W2 h) -- the
                # host ships g64 = G*32 (bf16) and w2t = W2^T*64 (fp8)
                psE = psA.tile([HID, FQ], F32, tag="main", name="psE")
                nc.tensor.matmul(psE[:], g64_sb[:], r_sb[:, ts],
                                 start=True, stop=False)
                for t2 in range(C // 2):
                    nc.tensor.matmul(
                        psE[:], w2t_sb[:, 2 * t2:2 * t2 + 2, :],
                        hf8_sb[:, 2 * t2:2 * t2 + 2, ts],
                        start=False, stop=(t2 == C // 2 - 1),
                        perf_mode=DR,
                    )
                mu = vecp.tile([1, FQ], BF16, tag="v1")
                msq = vecp.tile([1, FQ], F32, tag="v3")
                nc.vector.tensor_scalar_mul(mu[:], ps1[:], 1.0 / D)
                nc.gpsimd.partition_broadcast(mu1_bc[:, ts], mu[0:1, :])
                nc.scalar.activation(msq[:], ps1[:],
                                     mybir.ActivationFunctionType.Square,
                                     scale=1.0 / D)
                nc.vector.scalar_tensor_tensor(
                    e_sb[:, ts], psE[:], 1.0 / (WS / 2.0), r_sb[:, ts],
                    MULT, MULT)
                return msq

            def fused_ln_stats_b(nq, li, msq):
                """Part b: colsum(e) -> alpha chain -> rstd broadcast.
                Layer 0 accumulates the exact precomputed sum(x^2) instead
                of relying on the unit-variance identity."""
                ts = slice(nq * FQ, (nq + 1) * FQ)
                ps2 = stat_tile(FQ)
                nc.tensor.matmul(ps2[:], ones_bf[:HID, :], e_sb[:, ts],
                                 start=True, stop=(li > 0))
                if li == 0:
                    nc.tensor.matmul(ps2[:], ones_bf[0:1, :],
                                     xsqs_sb[0:1, ts],
                                     start=False, stop=True)
                ev = vecp.tile([1, FQ], F32, tag="v2")
                alpha = vecp.tile([1, FQ], BF16, tag="v6")
                nc.vector.scalar_tensor_tensor(
                    ev[:], ps2[:], 1.0 / D, msq[:], MULT, SUB)
                nc.scalar.activation(ev[:], ev[:], LN_,
                                     bias=eps2_sb[:] if li == 0 else onep_sb[:],
                                     scale=1.0 + EPS)
                nc.scalar.activation(alpha[:], ev[:], EXP, scale=-0.5)
                nc.gpsimd.partition_broadcast(rstd1_bc[:, ts], alpha[0:1, :])

            def fused_ln_norm(t0, tw, out_last=False):
                """Apply h = (t - mu)*alpha using the precomputed
                broadcasts. hf8 (chunks 0-1 first) gates the next layer's
                scores, so it is written before the bf16 h."""
                ts = slice(t0, t0 + tw)
                bshape = (P, C, tw)
                bs2 = (P, 2, tw)
                nc.vector.tensor_tensor(
                    h_sb[:, :, ts], t_sb[:, :, ts],
                    mu1_bc[:, None, ts].to_broadcast(bshape), SUB,
                )
                if out_last:
                    nc.vector.tensor_tensor(
                        hout_sb[:, :, ts], h_sb[:, :, ts],
                        rstd1_bc[:, None, ts].to_broadcast(bshape), MULT,
                    )
                    store_eng = [nc.sync, nc.scalar, nc.gpsimd, nc.sync]
                    for c in range(C):
                        store_eng[c].dma_start(out[c, :, ts],
                                               hout_sb[:, c, ts])
                else:
                    # hf8 chunk-pair 0-1 first: the next layer's first
                    # scores matmul needs only those chunks
                    for h2 in range(2):
                        nc.vector.tensor_tensor(
                            hf8_sb[:, 2 * h2:2 * h2 + 2, ts],
                            h_sb[:, 2 * h2:2 * h2 + 2, ts],
                            rstd1_bc[:, None, ts].to_broadcast(bs2), MULT,
                        )
                    nc.vector.tensor_tensor(
                        h_sb[:, :, ts], h_sb[:, :, ts],
                        rstd1_bc[:, None, ts].to_broadcast(bshape), MULT,
                    )

            # ---- transformer layers: flat chunk-step pipeline ----
            steps = [(li, nq) for li in range(L) for nq in range(NQ)]
            pending_ln = []

            fast = trivial_bias and trivial_ln
            emit_scores(steps[0][1])
            # per-token sum(x), sum(x^2) for the layer-0 LN stats; sits
            # behind the first scores block so the late h DMA is covered
            if fast:
                for nqq in range(NQ):
                    tsx = slice(nqq * FQ, (nqq + 1) * FQ)
                    nc.vector.tensor_mul(tsq_sb[:, :, tsx], h_sb[:, :, tsx],
                                         h_sb[:, :, tsx])
                    psx = stat_tile(FQ)
                    for c in range(C):
                        nc.tensor.matmul(psx[:], ones_bf[:], h_sb[:, c, tsx],
                                         start=(c == 0), stop=(c == C - 1))
                    nc.vector.tensor_copy(xsum_sb[0:1, tsx], psx[:])
                    psx2 = stat_tile(FQ)
                    for c in range(C):
                        nc.tensor.matmul(psx2[:], ones_bf[:],
                                         tsq_sb[:, c, tsx],
                                         start=(c == 0), stop=(c == C - 1))
                    nc.vector.tensor_copy(xsqs_sb[0:1, tsx], psx2[:])
            for i, (li, nq) in enumerate(steps):
                last = li == L - 1
                # previous step's pending work flushes here: the layer-0 LN
                # (or general-path LN) DVE chain hides under attn+scores
                while pending_ln:
                    pending_ln.pop(0)()

                if fast:
                    # merged steady-state: next step's scores groups
                    # interleave with this step's attn chunks so the exp
                    # chain starts ~4us earlier and attn matmuls fill its
                    # wait slots; LN stats (f(r, hf8) only) and ff1 slot
                    # into the scores tail
                    final = i == len(steps) - 1
                    if not final:
                        for u in range(4):
                            attn_chunk(nq, u)
                        emit_ff1(nq)
                        nxt = steps[i + 1][1]
                        psd = scores_begin()
                        msq = None
                        for g in range(MK // 2):
                            scores_group(nxt, g, psd)
                            if g == 2:
                                msq = fused_ln_stats_a(nq, li)
                            elif g == 5:
                                fused_ln_stats_b(nq, li, msq)
                        scores_end(nxt, psd)
                        emit_ff2(nq, li, inline_norm=True, out_last=last)
                    else:
                        for u in range(4):
                            attn_chunk(nq, u)
                        emit_ff1(nq)
                        msq = fused_ln_stats_a(nq, li)
                        fused_ln_stats_b(nq, li, msq)
                        # final step: ff2 + normalize + store in
                        # half-windows so the tail drains incrementally
                        sw = FQ // 2
                        for j in range(2):
                            emit_ff2(nq, li, nq * FQ + j * sw, sw,
                                     inline_norm=True, out_last=True)
                    continue

                emit_attn(nq)
                emit_ff1(nq)
                if i + 1 < len(steps):
                    emit_scores(steps[i + 1][1])
                emit_ff2(nq, li)

                def _ln(nq=nq, last=last, li=li):
                    if trivial_ln:
                        fused_ln_stats_l0(nq * FQ, FQ)
                        fused_ln_norm(nq * FQ, FQ, out_last=last)
                    else:
                        layer_norm(t_sb, t_sb, g1_sb, be1_sb,
                                   mu1_bc, rstd1_bc, nq, use_stw=True)
                        layer_norm(t_sb, h_sb, g2_sb, be2_sb,
                                   mu2_bc, rstd2_bc, nq, out_last=last)
                pending_ln.append(_ln)
            while pending_ln:
                pending_ln.pop(0)()
    nc.compile()
    return nc


_NC_CACHE = {}


def _get_nc(trivial_ln, trivial_bias=False):
    key = ("nc", trivial_ln, trivial_bias)
    if key not in _NC_CACHE:
        _NC_CACHE[key] = build(trivial_ln=trivial_ln, trivial_bias=trivial_bias)
    return _NC_CACHE[key]


def _shard_inputs(x, Wq, bq, Wk, bk_, Wv, bv_, W1, b1, W2, b2, ln1_g, ln1_b, ln2_g, ln2_b):
    """Full inputs -> list of 8 per-core in_maps."""
    bf = ml_dtypes.bfloat16
    f8 = ml_dtypes.float8_e4m3
    C = D // P
    SQ = S // 2
    shared = {
        "wq": np.ascontiguousarray(Wq.T * WS).reshape(C, P, D).astype(f8),
        "wk": np.ascontiguousarray(Wk * WS).reshape(C, P, D).astype(f8),
        "wv": np.ascontiguousarray(Wv * WS).reshape(C, P, D).astype(f8),
        "w1": np.ascontiguousarray(W1 * WS).reshape(C, P, HID).astype(f8),
        "w2": np.ascontiguousarray(W2).astype(bf),
        "w2c": np.ascontiguousarray(W2.sum(axis=1).reshape(HID, 1)).astype(bf),
        "w2t": np.ascontiguousarray(W2.T * WS).reshape(C, P, HID).astype(f8),
        "g64": np.ascontiguousarray((W2 @ W2.T) * (WS / 2.0)).astype(bf),
        "bqc": np.ascontiguousarray(bq.reshape(C, P, 1)).astype(f8),
        "bk": np.ascontiguousarray(bk_.reshape(C, P).T).astype(np.float32),
        "bv": np.ascontiguousarray(bv_.reshape(1, D) * VS).astype(np.float32),
        "b1d": np.ascontiguousarray(b1.reshape(HID, 1)).astype(np.float32),
        "b2r": np.ascontiguousarray(b2.reshape(1, D)).astype(bf),
        "g1d": np.ascontiguousarray(ln1_g.reshape(C, P).T).astype(np.float32),
        "be1d": np.ascontiguousarray(ln1_b.reshape(C, P).T).astype(np.float32),
        "g2d": np.ascontiguousarray(ln2_g.reshape(C, P).T).astype(np.float32),
        "be2d": np.ascontiguousarray(ln2_b.reshape(C, P).T).astype(np.float32),
    }
    in_maps = []
    for core in range(8):
        b, j = core // 2, core % 2
        xT = np.ascontiguousarray(x[b].T)  # [D, S]
        xT8 = xT.astype(f8)
        m = dict(shared)
        m["xt"] = xT8.reshape(C, P, S)
        m["xq"] = np.ascontiguousarray(
            xT[:, j * SQ:(j + 1) * SQ].reshape(C, P, SQ)
        ).astype(bf)
        m["xq8"] = np.ascontiguousarray(
            xT8.reshape(C, P, S)[:, :, j * SQ:(j + 1) * SQ])
        in_maps.append(m)
    return in_maps


def _gather_output(results):
    SQ = S // 2
    out = np.empty((B, S, D), np.float32)
    for core, res in enumerate(results):
        b, j = core // 2, core % 2
        # res["out"]: [C, P, SQ] = h^T chunks -> h slice [SQ, D]
        out[b, j * SQ:(j + 1) * SQ, :] = (
            res["out"].astype(np.float32).reshape(D, SQ).T)
    return out


def _ln_trivial(inputs):
    return bool(
        np.all(inputs["ln1_g"] == 1.0) and np.all(inputs["ln1_b"] == 0.0)
        and np.all(inputs["ln2_g"] == 1.0) and np.all(inputs["ln2_b"] == 0.0)
    )


def _bias_trivial(inputs):
    return bool(all(np.all(inputs[k] == 0.0) for k in ("bq", "b2", "bv")))


def kernel(**inputs):
    nc = _get_nc(trivial_ln=_ln_trivial(inputs), trivial_bias=_bias_trivial(inputs))
    in_maps = _shard_inputs(
        inputs["x"], inputs["Wq"], inputs["bq"], inputs["Wk"], inputs["bk"],
        inputs["Wv"], inputs["bv"], inputs["W1"], inputs["b1"], inputs["W2"],
        inputs["b2"], inputs["ln1_g"], inputs["ln1_b"], inputs["ln2_g"],
        inputs["ln2_b"],
    )
    res = run_bass_kernel_spmd(nc, in_maps, core_ids=list(range(8)))
    return _gather_output(res.results)


# revision 42
# speedup vs baseline: 1.0127x; 1.0117x over previous
"""Blockwise-parallel transformer attention on 8 TRN2 NeuronCores.

Reference computation (per batch b):
    k = x@Wk + bk ; v = x@Wv + bv            (from ORIGINAL x, layer-invariant)
    h = x
    6x (shared weights):
        q = h@Wq + bq
        P = softmax(q k^T / 8)
        attn = (P @ v) / sqrt(512)
        ff = relu(attn@W1 + b1)@W2 + b2
        h = LN2(LN1(h + ff))

Sharding: 8 cores = 4 batches x 2 query-halves. Each core computes full
k/v for its batch (once), then processes its 1024-query slice through all
6 layers with zero cross-core traffic.

On-chip layout is fully transposed (feature dim on partitions, tokens on
the free axis); the host feeds x^T so the device never transposes.

Key algebraic move: scores = (h Wq + bq) k^T = h (k Wq^T)^T + bq k^T.
k is layer-invariant, so k' = k Wq^T is computed ONCE at setup and the
per-layer q projection disappears entirely; bq folds into the exp bias
via ck = (k@bq)/8. The residual add rides in the PSUM->SBUF copy
(DVE tensor_tensor) instead of an identity matmul.

fp8 numerics: fp8 weights are host-prescaled x64 (values ~0.02 would
otherwise sit in the fp8e4m3 subnormal range) and rescaled by 1/64 in
the PSUM->SBUF copies; v is stored x16 for the same reason. ff1 runs
fp8 DoubleRow on attn/1024 with W1*64.

Trivial-path (g=1,b=0,zero biases) specializations: LN2(LN1(t)) fuses
to (t-mu1)*alpha; softmax exps merge pairwise over 2-bank PSUM tiles
(ACT cost is per-column, so this nearly halves the exp chain); the
softmax-denominator partial sums ride the exp-wait slots of the scores
stream, one step ahead of their consumer; the LN statistics are derived
algebraically from (r, hf8) alone -- colsum(t) = w2c@r since the LN
output is zero-mean, colsum(t^2) = r^T(W2 W2^T)r + 2r^T(W2 h) + D since
it is unit-variance (layer 0 uses precomputed sum(x), sum(x^2)) -- so
they run interleaved in the scores stream two steps ahead of the
normalize, which itself rides chunk-pair-wise behind the ff2 residual
adds. Per-iteration PE order: attn(s), ff1(s), scores(s+1) x8 groups
(with denominator partials + LN stats of s in the gaps), ff2(s) with
inline normalize. The final step emits ff2+normalize+store in
half-windows so the tail drains incrementally.
"""

import sys

if "/opt/trn_rl_repo" not in sys.path:
    sys.path.insert(0, "/opt/trn_rl_repo")

import numpy as np
import ml_dtypes

import concourse.bass as bass
import concourse.mybir as mybir
import concourse.tile as tile
from concourse import bacc
import concourse.hw_specs as _hw_specs


def _restrict_act_tables():
    """All activation functions this kernel uses (exp, ln, relu, copy)
    live in the natural_log_exp_and_others table set. Left to its own
    devices the table-load pass alternates between exp_and_others and the
    ln set (~49 reloads x 1.5us of ACT time per run); restricting the
    offered sets collapses that to a single load. Dict order is preserved
    so act_func_set_id stays aligned with act_info.json."""
    if getattr(_hw_specs, "_act_tables_restricted", False):
        return
    orig = _hw_specs.get_activation_tables

    def restricted(arch):
        tables = orig(arch)
        return {
            name: (fns if name == "natural_log_exp_and_others" else set())
            for name, fns in tables.items()
        }

    _hw_specs.get_activation_tables = restricted
    bacc.get_activation_tables = restricted
    _hw_specs._act_tables_restricted = True


_restrict_act_tables()
from concourse.bass_utils import run_bass_kernel_spmd
from concourse.masks import make_identity

F32 = mybir.dt.float32
BF16 = mybir.dt.bfloat16
F8 = mybir.dt.float8e4
DR = mybir.MatmulPerfMode.DoubleRow
EXP = mybir.ActivationFunctionType.Exp
LN_ = mybir.ActivationFunctionType.Ln
RELU = mybir.ActivationFunctionType.Relu
IDENT = mybir.ActivationFunctionType.Identity
ADD = mybir.AluOpType.add
SUB = mybir.AluOpType.subtract
MULT = mybir.AluOpType.mult

B, S, D, HID, L = 4, 2048, 512, 64, 6
EPS = 1e-5
P = 128
WS = 64.0   # fp8 weight prescale (host x64, on-chip 1/64)
VS = 16.0   # v prescale


def build(S=S, SQ=S // 2, D=D, HID=HID, L=L, trivial_ln=False, trivial_bias=False):
    """Build + compile the per-core Bass program (same program on all 8 cores)."""
    C = D // P          # feature-dim 128-chunks (4)
    MK = S // P         # key-token 128-chunks (16)
    FK = min(512, S)    # key free-dim tile
    NK = S // FK
    FQ = min(512, SQ)   # query free-dim tile
    NQ = SQ // FQ
    scale_attn = 1.0 / float(np.sqrt(HID))
    scale_out = 1.0 / float(np.sqrt(D))

    nc = bacc.Bacc("TRN2", target_bir_lowering=False, debug=False)

    # ---- DRAM I/O (per core) ----
    xt = nc.dram_tensor("xt", (C, P, S), F8, kind="ExternalInput")
    xq = nc.dram_tensor("xq", (C, P, SQ), BF16, kind="ExternalInput")
    xq8 = nc.dram_tensor("xq8", (C, P, SQ), F8, kind="ExternalInput")
    wqT = nc.dram_tensor("wq", (C, P, D), F8, kind="ExternalInput")   # Wq^T x64
    wk = nc.dram_tensor("wk", (C, P, D), F8, kind="ExternalInput")    # Wk x64
    wv = nc.dram_tensor("wv", (C, P, D), F8, kind="ExternalInput")    # Wv x64
    w1 = nc.dram_tensor("w1", (C, P, HID), F8, kind="ExternalInput")  # W1 x64
    w2 = nc.dram_tensor("w2", (HID, D), BF16, kind="ExternalInput")
    w2c = nc.dram_tensor("w2c", (HID, 1), BF16, kind="ExternalInput")
    w2t = nc.dram_tensor("w2t", (C, P, HID), F8, kind="ExternalInput")  # W2^T x64
    g64 = nc.dram_tensor("g64", (HID, HID), BF16, kind="ExternalInput")  # W2 W2^T
    bqc = nc.dram_tensor("bqc", (C, P, 1), F8, kind="ExternalInput")
    bk = nc.dram_tensor("bk", (P, C), F32, kind="ExternalInput")
    bv = nc.dram_tensor("bv", (1, D), F32, kind="ExternalInput")      # bv x16
    b1d = nc.dram_tensor("b1d", (HID, 1), F32, kind="ExternalInput")
    b2r = nc.dram_tensor("b2r", (1, D), BF16, kind="ExternalInput")
    g1d = nc.dram_tensor("g1d", (P, C), F32, kind="ExternalInput")
    be1d = nc.dram_tensor("be1d", (P, C), F32, kind="ExternalInput")
    g2d = nc.dram_tensor("g2d", (P, C), F32, kind="ExternalInput")
    be2d = nc.dram_tensor("be2d", (P, C), F32, kind="ExternalInput")
    out = nc.dram_tensor("out", (C, P, SQ), BF16, kind="ExternalOutput")

    # PSUM bank budget (8 banks):
    #  trivial: psA x4 [P,FQ] (attn/ff) + psP x2 of 2 banks (score pairs,
    #           LN stats and softmax denominator share psP's rotation)
    #  general: psA x6 + psS x2 (baseline layout)
    nA = 4 if trivial_bias else 6
    with tile.TileContext(nc) as tc:
        with (
            tc.tile_pool(name="const", bufs=1) as cons,
            tc.tile_pool(name="big", bufs=1) as big,
            tc.tile_pool(name="vec", bufs=2) as vecp,
            tc.tile_pool(name="psA", bufs=nA, space="PSUM") as psA,
            tc.tile_pool(name="psS", bufs=2, space="PSUM") as psS,
            tc.tile_pool(name="psP", bufs=2, space="PSUM") as psP,
        ):
            def stat_tile(tw):
                if trivial_bias:
                    return psP.tile([1, tw], F32, tag="pp", name="pstat")
                return psS.tile([1, tw], F32, tag="stat", name="pstat")

            # ---- persistent SBUF ----
            wqT_sb = cons.tile([P, C, D], F8)
            wk_sb = cons.tile([P, C, D], F8)
            wv_sb = cons.tile([P, C, D], F8)
            w1_sb = cons.tile([P, C, HID], F8)
            w2_sb = cons.tile([HID, D], BF16)
            w2c_sb = cons.tile([HID, 1], BF16)
            w2t_sb = cons.tile([P, C, HID], F8)
            g64_sb = cons.tile([HID, HID], BF16)
            e_sb = cons.tile([HID, SQ], BF16)   # r*(Gr + 2 W2 h) for var stats
            xsum_sb = cons.tile([1, SQ], BF16)  # per-token sum(x) (layer 0)
            xsqs_sb = cons.tile([1, SQ], BF16)  # per-token sum(x^2) (layer 0)
            onep_sb = cons.tile([1, 1], F32)    # 1 + eps + eps^2
            bqc_sb = cons.tile([P, C], F8)
            bk_sb = cons.tile([P, C], F32)
            bv_sb = cons.tile([1, D], F32)
            bv_bc = cons.tile([P, D], F32)
            b1_sb = cons.tile([HID, 1], F32)
            b2r_sb = cons.tile([1, D], BF16)
            g1_sb = cons.tile([P, C], F32)
            be1_sb = cons.tile([P, C], F32)
            g2_sb = cons.tile([P, C], F32)
            be2_sb = cons.tile([P, C], F32)
            ones_bf = cons.tile([P, 1], BF16)
            ones2_f8 = cons.tile([P, 2, 16], F8)  # pair-dim stride must be 16B
            ones_row = cons.tile([1, SQ], BF16)
            eps_sb = cons.tile([1, 1], F32)
            eps2_sb = cons.tile([1, 1], F32)
            ident_sb = cons.tile([P, P], BF16)
            ck_sb = cons.tile([P, MK], F32)   # exp bias: (k @ bq)/8 per key token

            k_sb = cons.tile([P, C, S], F8)       # k^T (fp8)
            kq_sb = cons.tile([P, C, S], F8)      # k'^T = Wq k^T (fp8)
            v_sb = cons.tile([P, MK, D], F8)      # v natural x16 (fp8)
            vbf_sb = None
            if not trivial_bias:
                vbf_sb = cons.tile([P, MK, D], BF16)
            h_sb = cons.tile([P, C, SQ], BF16)    # h^T (residual stream)
            hf8_sb = cons.tile([P, C, SQ], F8)    # h^T in fp8 for the scores matmul
            attn8_sb = cons.tile([P, C, SQ], F8)  # attn^T / (WS*VS) in fp8
            r_sb = cons.tile([HID, SQ], BF16)     # relu(ffn hidden)
            t_sb = cons.tile([P, C, SQ], BF16)    # residual pre-LN
            tsq_sb = cons.tile([P, C, SQ], BF16)
            stw_sb = None
            if not trivial_ln:
                stw_sb = cons.tile([P, 2, SQ], BF16)  # [sum(t), sum(t^2)]
            hout_sb = cons.tile([P, C, SQ], BF16)  # final-layer output
            recip_bc = cons.tile([P, SQ], F32)
            mu1_bc = cons.tile([P, SQ], BF16)
            rstd1_bc = cons.tile([P, SQ], BF16)
            mu2_bc = cons.tile([P, SQ], BF16)
            rstd2_bc = cons.tile([P, SQ], BF16)

            # fp8 P is safe only with zero biases (logits stay in ~[-3,3]);
            # the general path keeps bf16 P and standard matmuls
            P_dt = F8 if trivial_bias else BF16
            P_sb = cons.tile([P, MK, SQ], P_dt)  # exp(scores^T)

            xt_sb = big.tile([P, C, S], F8, tag="bigshare")

            # ---- constants first so the PE warmup isn't gated on queue
            # drain, then inputs spread across engine queues (each dma_start
            # costs ~600ns of issue time on its queue) ----
            nc.vector.memset(ones_bf[:], 1.0)
            nc.vector.memset(ones2_f8[:], 1.0)
            nc.vector.memset(eps_sb[:], EPS)
            nc.vector.memset(eps2_sb[:], EPS * EPS)
            nc.vector.memset(onep_sb[:], 1.0 + EPS + EPS * EPS)
            if not trivial_bias:
                nc.vector.memset(ones_row[:], 1.0)
            # the warmup only needs SOME valid operand data, not a real
            # identity (nothing else reads ident_sb): a cheap DVE memset
            # unblocks the PE ~2us earlier than the gpsimd iota ucode
            nc.vector.memset(ident_sb[:], 1.0)

            for c in range(C):
                nc.sync.dma_start(wk_sb[:, c, :], wk[c, :, :])
            for c in range(C):
                nc.sync.dma_start(xt_sb[:, c, 0:S // 2], xt[c, :, 0:S // 2])
            for c in range(C):
                nc.sync.dma_start(xt_sb[:, c, S // 2:S], xt[c, :, S // 2:S])
            nc.gpsimd.dma_start(bk_sb[:], bk[:, :])
            for c in range(C):
                nc.gpsimd.dma_start(wqT_sb[:, c, :], wqT[c, :, :])
            for c in range(C):
                nc.gpsimd.dma_start(wv_sb[:, c, :], wv[c, :, :])
            nc.gpsimd.dma_start(w1_sb[:], w1[:, :, :].rearrange("c p d -> p c d"))
            nc.gpsimd.dma_start(w2_sb[:], w2[:, :])
            nc.gpsimd.dma_start(w2c_sb[:], w2c[:, :])
            nc.gpsimd.dma_start(w2t_sb[:], w2t[:, :, :].rearrange("c p d -> p c d"))
            nc.gpsimd.dma_start(g64_sb[:], g64[:, :])
            for c in range(C):
                nc.scalar.dma_start(hf8_sb[:, c, :], xq8[c, :, :])
            nc.gpsimd.dma_start(b1_sb[:], b1d[:, :])
            if not trivial_bias:
                nc.gpsimd.dma_start(bqc_sb[:], bqc[:, :, 0].rearrange("c p -> p c"))
                nc.gpsimd.dma_start(bv_sb[:], bv[:, :])
                nc.gpsimd.dma_start(b2r_sb[:], b2r[:, :])
            if not trivial_ln:
                nc.gpsimd.dma_start(g1_sb[:], g1d[:, :])
                nc.gpsimd.dma_start(be1_sb[:], be1d[:, :])
                nc.gpsimd.dma_start(g2_sb[:], g2d[:, :])
                nc.gpsimd.dma_start(be2_sb[:], be2d[:, :])
            if not trivial_bias:
                nc.gpsimd.partition_broadcast(bv_bc[:], bv_sb[0:1, :])

            # HAM warmup: keep the PE busy while the input DMAs land so the
            # first real matmuls run at full clock
            wu = psA.tile([P, P], F32, tag="main")
            for _ in range(80):
                nc.tensor.matmul(wu[:], ident_sb[:], ident_sb[:],
                                 start=True, stop=True)

            # ---- k^T = (Wk^T x^T)/WS + bk ;  k'^T = (Wq k^T)/WS ----
            def k_tile(nk):
                for c in range(C):
                    ps = psA.tile([P, FK], F32, tag="main")
                    for t2 in range(C // 2):
                        nc.tensor.matmul(
                            ps[:],
                            wk_sb[:, 2 * t2:2 * t2 + 2, c * P:(c + 1) * P],
                            xt_sb[:, 2 * t2:2 * t2 + 2, nk * FK:(nk + 1) * FK],
                            start=(t2 == 0),
                            stop=(t2 == C // 2 - 1),
                            perf_mode=DR,
                        )
                    nc.scalar.activation(
                        k_sb[:, c, nk * FK:(nk + 1) * FK], ps[:], IDENT,
                        bias=bk_sb[:, c:c + 1], scale=1.0 / WS,
                    )

            def kq_tile(nk):
                for c in range(C):
                    ps = psA.tile([P, FK], F32, tag="main")
                    for t2 in range(C // 2):
                        nc.tensor.matmul(
                            ps[:],
                            wqT_sb[:, 2 * t2:2 * t2 + 2, c * P:(c + 1) * P],
                            k_sb[:, 2 * t2:2 * t2 + 2, nk * FK:(nk + 1) * FK],
                            start=(t2 == 0),
                            stop=(t2 == C // 2 - 1),
                            perf_mode=DR,
                        )
                    if c % 2 == 0:
                        nc.scalar.activation(
                            kq_sb[:, c, nk * FK:(nk + 1) * FK], ps[:], IDENT,
                            scale=1.0 / WS)
                    else:
                        nc.vector.tensor_scalar_mul(
                            kq_sb[:, c, nk * FK:(nk + 1) * FK], ps[:], 1.0 / WS)

            k_tile(0)
            k_tile(1)
            kq_tile(0)
            k_tile(2)
            kq_tile(1)
            k_tile(3)
            kq_tile(2)
            kq_tile(3)

            # h (bf16) is needed first by the layer-0 x-stats (~20us in);
            # its DMA issues late so it doesn't compete with the k-path
            for c in range(C):
                nc.gpsimd.dma_start(h_sb[:, c, :], xq[c, :, :])



            # ---- ck = (k @ bq) * scale_attn  (exp bias; layer-invariant) ----
            for mk in range(MK) if not trivial_bias else []:
                ps = psS.tile([P, 1], F32, tag="stat")
                for c in range(C):
                    nc.tensor.matmul(
                        ps[:],
                        k_sb[:, c, mk * P:(mk + 1) * P],
                        bqc_sb[:, c:c + 1],
                        start=(c == 0),
                        stop=(c == C - 1),
                    )
                nc.vector.tensor_scalar_mul(ck_sb[:, mk:mk + 1], ps[:], scale_attn)

            # ---- v = (x@Wv)*VS/WS + bv*VS ----
            for mk in range(MK):
                ps = psA.tile([P, D], F32, tag="main")
                for t2 in range(C // 2):
                    nc.tensor.matmul(
                        ps[:],
                        xt_sb[:, 2 * t2:2 * t2 + 2, mk * P:(mk + 1) * P],
                        wv_sb[:, 2 * t2:2 * t2 + 2, :],
                        start=(t2 == 0),
                        stop=(t2 == C // 2 - 1),
                        perf_mode=DR,
                    )
                if trivial_bias:
                    # bv == 0 in the trivial path: plain scaled copy
                    nc.vector.tensor_scalar_mul(v_sb[:, mk, :], ps[:], VS / WS)
                else:
                    nc.vector.scalar_tensor_tensor(
                        v_sb[:, mk, :], ps[:], VS / WS, bv_bc[:], MULT, ADD)
                    nc.vector.scalar_tensor_tensor(
                        vbf_sb[:, mk, :], ps[:], VS / WS, bv_bc[:], MULT, ADD)

            # ---- per-chunk pipeline pieces ----
            def scores_begin():
                # softmax-denominator accumulator; its partials ride the
                # exp-wait slots of the 2-deep psP rotation
                return psA.tile([1, FQ], F32, tag="main", name="psd")

            def scores_group(nq, mg, psd):
                # one pair of key chunks shares a 2-bank PSUM tile -> one
                # [128, 2*FQ] exp per pair (ACT cost is per-column)
                ts = slice(nq * FQ, (nq + 1) * FQ)
                pp = psP.tile([P, 2 * FQ], F32, tag="pp")
                for half in range(2):
                    mk = 2 * mg + half
                    for t2 in range(C // 2):
                        nc.tensor.matmul(
                            pp[:, half * FQ:(half + 1) * FQ],
                            kq_sb[:, 2 * t2:2 * t2 + 2,
                                  mk * P:(mk + 1) * P],
                            hf8_sb[:, 2 * t2:2 * t2 + 2, ts],
                            start=(t2 == 0),
                            stop=(t2 == C // 2 - 1),
                            perf_mode=DR,
                        )
                nc.scalar.activation(
                    P_sb[:, 2 * mg:2 * mg + 2, ts], pp[:], EXP,
                    bias=0.0, scale=scale_attn)
                nc.tensor.matmul(
                    psd[:], ones2_f8[:, :, 0:1],
                    P_sb[:, 2 * mg:2 * mg + 2, ts],
                    start=(mg == 0), stop=(mg == MK // 2 - 1),
                    perf_mode=DR,
                )

            def scores_end(nq, psd):
                ts = slice(nq * FQ, (nq + 1) * FQ)
                den = vecp.tile([1, FQ], F32, tag="vden")
                nc.vector.reciprocal_approx_fast(den[:], psd[:])
                nc.gpsimd.partition_broadcast(recip_bc[:, ts], den[0:1, :])

            def attn_chunk(nq, c):
                # attn^T chunk c = v^T P^T (x VS)
                ts = slice(nq * FQ, (nq + 1) * FQ)
                ps = psA.tile([P, FQ], F32, tag="main", name="psa")
                for t2 in range(MK // 2):
                    nc.tensor.matmul(
                        ps[:],
                        v_sb[:, 2 * t2:2 * t2 + 2, c * P:(c + 1) * P],
                        P_sb[:, 2 * t2:2 * t2 + 2, ts],
                        start=(t2 == 0),
                        stop=(t2 == MK // 2 - 1),
                        perf_mode=DR,
                    )
                nc.scalar.activation(attn8_sb[:, c, ts], ps[:], IDENT,
                                     scale=1.0 / (WS * VS))

            def emit_scores(nq):
                # standalone form (pre-loop / general path)
                ts = slice(nq * FQ, (nq + 1) * FQ)
                if trivial_bias:
                    psd = scores_begin()
                    for mg in range(MK // 2):
                        scores_group(nq, mg, psd)
                    scores_end(nq, psd)
                else:
                    for mk in range(MK):
                        ps = psA.tile([P, FQ], F32, tag="main")
                        for t2 in range(C // 2):
                            nc.tensor.matmul(
                                ps[:],
                                kq_sb[:, 2 * t2:2 * t2 + 2, mk * P:(mk + 1) * P],
                                hf8_sb[:, 2 * t2:2 * t2 + 2, ts],
                                start=(t2 == 0),
                                stop=(t2 == C // 2 - 1),
                                perf_mode=DR,
                            )
                        nc.scalar.activation(
                            P_sb[:, mk, ts], ps[:], EXP,
                            bias=ck_sb[:, mk:mk + 1], scale=scale_attn)

            def emit_attn(nq):
                # attn^T = v^T P^T (x VS); denominator colsum after c==0
                ts = slice(nq * FQ, (nq + 1) * FQ)
                for c in range(C):
                    ps = psA.tile([P, FQ], F32, tag="main")
                    if trivial_bias:
                        for t2 in range(MK // 2):
                            nc.tensor.matmul(
                                ps[:],
                                v_sb[:, 2 * t2:2 * t2 + 2, c * P:(c + 1) * P],
                                P_sb[:, 2 * t2:2 * t2 + 2, ts],
                                start=(t2 == 0),
                                stop=(t2 == MK // 2 - 1),
                                perf_mode=DR,
                            )
                    else:
                        for mk in range(MK):
                            nc.tensor.matmul(
                                ps[:],
                                vbf_sb[:, mk, c * P:(c + 1) * P],
                                P_sb[:, mk, ts],
                                start=(mk == 0),
                                stop=(mk == MK - 1),
                            )
                    if c == 0 and not trivial_bias:
                        psd = stat_tile(FQ)
                        for mk in range(MK):
                            nc.tensor.matmul(
                                psd[:], ones_bf[:], P_sb[:, mk, ts],
                                start=(mk == 0),
                                stop=(mk == MK - 1),
                            )
                        den = vecp.tile([1, FQ], F32, tag="vden")
                        nc.vector.reciprocal_approx_fast(den[:], psd[:])
                        nc.gpsimd.partition_broadcast(
                            recip_bc[:, ts], den[0:1, :])
                    nc.scalar.activation(attn8_sb[:, c, ts], ps[:], IDENT,
                                         scale=1.0 / (WS * VS))

            def emit_ff1(nq):
                # ffn hidden: psum = attn@W1 (fp8 DR); softmax recip + relu
                # applied on the [64 x FQ] hidden (recip commutes through W1)
                ts = slice(nq * FQ, (nq + 1) * FQ)
                ps = psA.tile([HID, FQ], F32, tag="main")
                for t2 in range(C // 2):
                    nc.tensor.matmul(
                        ps[:], w1_sb[:, 2 * t2:2 * t2 + 2, :],
                        attn8_sb[:, 2 * t2:2 * t2 + 2, ts],
                        start=(t2 == 0), stop=(t2 == C // 2 - 1),
                        perf_mode=DR,
                    )
                nc.vector.tensor_mul(ps[:], ps[:], recip_bc[:HID, ts])
                nc.scalar.activation(
                    r_sb[:, ts], ps[:], RELU,
                    bias=b1_sb[:, 0:1], scale=scale_out,
                )

            def emit_ff2(nq, li, t0=None, tw=None, inline_norm=False,
                         out_last=False):
                # ff2 + residual: t = W2^T r (+ b2) + h, add fused into the
                # PSUM->SBUF copy on DVE. t^2 is only materialized where the
                # LN stats can't be derived algebraically (layer 0 / general).
                # inline_norm: mu/alpha broadcasts are already out (stats ran
                # mid-iteration), so the normalize rides chunk-pair-wise right
                # behind the t-adds -- hf8 chunks 0-1 land early enough that
                # the next layer's scores never wait
                if t0 is None:
                    t0, tw = nq * FQ, FQ
                ts = slice(t0, t0 + tw)
                need_tsq = not (trivial_ln and trivial_bias)
                bs2 = (P, 2, tw)
                for c in range(C):
                    ps = psA.tile([P, tw], F32, tag="main", name="psf")
                    nc.tensor.matmul(
                        ps[:], w2_sb[:, c * P:(c + 1) * P], r_sb[:, ts],
                        start=True, stop=trivial_bias,
                    )
                    if not trivial_bias:
                        nc.tensor.matmul(
                            ps[:], b2r_sb[0:1, c * P:(c + 1) * P],
                            ones_row[0:1, ts], start=False, stop=True,
                        )
                    nc.vector.tensor_tensor(t_sb[:, c, ts], ps[:],
                                            h_sb[:, c, ts], ADD)
                    if need_tsq:
                        nc.vector.tensor_mul(tsq_sb[:, c, ts], t_sb[:, c, ts],
                                             t_sb[:, c, ts])
                    if inline_norm and c % 2 == 1:
                        pc2 = slice(c - 1, c + 1)
                        nc.vector.tensor_tensor(
                            h_sb[:, pc2, ts], t_sb[:, pc2, ts],
                            mu1_bc[:, None, ts].to_broadcast(bs2), SUB,
                        )
                        if out_last:
                            nc.vector.tensor_tensor(
                                hout_sb[:, pc2, ts], h_sb[:, pc2, ts],
                                rstd1_bc[:, None, ts].to_broadcast(bs2),
                                MULT,
                            )
                            store_eng = [nc.sync, nc.scalar, nc.gpsimd,
                                         nc.sync]
                            for cc in (c - 1, c):
                                store_eng[cc].dma_start(
                                    out[cc, :, ts], hout_sb[:, cc, ts])
                        else:
                            nc.vector.tensor_tensor(
                                hf8_sb[:, pc2, ts], h_sb[:, pc2, ts],
                                rstd1_bc[:, None, ts].to_broadcast(bs2),
                                MULT,
                            )
                if inline_norm and not out_last:
                    nc.vector.tensor_tensor(
                        h_sb[:, :, ts], h_sb[:, :, ts],
                        rstd1_bc[:, None, ts].to_broadcast((P, C, tw)), MULT,
                    )
                    if not trivial_ln:
                        if c == 1:
                            nc.vector.tensor_tensor(
                                stw_sb[:, 0, ts], t_sb[:, 0, ts],
                                t_sb[:, 1, ts], ADD)
                            nc.vector.tensor_tensor(
                                stw_sb[:, 1, ts], tsq_sb[:, 0, ts],
                                tsq_sb[:, 1, ts], ADD)
                        if c == 3:
                            nc.vector.tensor_tensor(
                                stw_sb[:, 0, ts], stw_sb[:, 0, ts],
                                t_sb[:, 2, ts], ADD)
                            nc.vector.tensor_tensor(
                                stw_sb[:, 0, ts], stw_sb[:, 0, ts],
                                t_sb[:, 3, ts], ADD)
                            nc.vector.tensor_tensor(
                                stw_sb[:, 1, ts], stw_sb[:, 1, ts],
                                tsq_sb[:, 2, ts], ADD)
                            nc.vector.tensor_tensor(
                                stw_sb[:, 1, ts], stw_sb[:, 1, ts],
                                tsq_sb[:, 3, ts], ADD)

            def layer_norm(src, dst, g, be, mu_bc, rstd_bc, nq, out_last=False,
                           use_stw=False):
                """General LN over the feature axis for token chunk nq.
                use_stw: the ff loop prebuilt sum(t)/sum(t^2) into stw_sb
                (valid for LN1 only; LN2 recomputes from its input)."""
                ts = slice(nq * FQ, (nq + 1) * FQ)
                if not use_stw:
                    nc.vector.tensor_mul(tsq_sb[:, :, ts], src[:, :, ts],
                                         src[:, :, ts])
                    nc.vector.tensor_tensor(
                        stw_sb[:, 0:1, ts], src[:, 0:1, ts], src[:, 1:2, ts], ADD)
                    nc.vector.tensor_tensor(
                        stw_sb[:, 0:1, ts], stw_sb[:, 0:1, ts], src[:, 2:3, ts], ADD)
                    nc.vector.tensor_tensor(
                        stw_sb[:, 0:1, ts], stw_sb[:, 0:1, ts], src[:, 3:4, ts], ADD)
                    nc.vector.tensor_tensor(
                        stw_sb[:, 1:2, ts], tsq_sb[:, 0:1, ts], tsq_sb[:, 1:2, ts], ADD)
                    nc.vector.tensor_tensor(
                        stw_sb[:, 1:2, ts], stw_sb[:, 1:2, ts], tsq_sb[:, 2:3, ts], ADD)
                    nc.vector.tensor_tensor(
                        stw_sb[:, 1:2, ts], stw_sb[:, 1:2, ts], tsq_sb[:, 3:4, ts], ADD)
                ps1 = stat_tile(FQ)
                nc.tensor.matmul(ps1[:], ones_bf[:], stw_sb[:, 0, ts],
                                 start=True, stop=True)
                ps2 = stat_tile(FQ)
                nc.tensor.matmul(ps2[:], ones_bf[:], stw_sb[:, 1, ts],
                                 start=True, stop=True)
                mu = vecp.tile([1, FQ], BF16, tag="v1")
                ev = vecp.tile([1, FQ], F32, tag="v2")
                msq = vecp.tile([1, FQ], F32, tag="v3")
                rstd = vecp.tile([1, FQ], BF16, tag="v4")
                nc.vector.tensor_scalar_mul(mu[:], ps1[:], 1.0 / D)
                nc.vector.tensor_scalar_mul(ev[:], ps2[:], 1.0 / D)
                nc.vector.tensor_mul(msq[:], mu[:], mu[:])
                nc.vector.tensor_tensor(ev[:], ev[:], msq[:], SUB)
                nc.scalar.activation(ev[:], ev[:], LN_, bias=eps_sb[:])
                nc.scalar.activation(rstd[:], ev[:], EXP, scale=-0.5)
                nc.gpsimd.partition_broadcast(mu_bc[:, ts], mu[0:1, :])
                nc.gpsimd.partition_broadcast(rstd_bc[:, ts], rstd[0:1, :])
                bshape = (P, C, FQ)
                nc.vector.tensor_tensor(
                    dst[:, :, ts], src[:, :, ts],
                    mu_bc[:, None, ts].to_broadcast(bshape), SUB,
                )
                nc.vector.tensor_tensor(
                    dst[:, :, ts], dst[:, :, ts],
                    rstd_bc[:, None, ts].to_broadcast(bshape), MULT,
                )
                dd = hout_sb if out_last else dst
                for c in range(C):
                    nc.vector.tensor_scalar(
                        dd[:, c, ts], dst[:, c, ts],
                        g[:, c:c + 1], be[:, c:c + 1], MULT, ADD,
                    )
                    if out_last:
                        nc.sync.dma_start(out[c, :, ts], hout_sb[:, c, ts])
                if not out_last and dst is not t_sb:
                    nc.vector.tensor_copy(hf8_sb[:, :, ts], dst[:, :, ts])

            def fused_ln_alpha(ps1, ps2, tw, ts, extra_var=0.0):
                """Shared mu/alpha tail: mu = ps1/D broadcast early; v1 =
                ps2/D - mu^2 + extra_var; alpha = rsqrt(v1(1+eps)+eps^2)
                via ln/exp. Broadcasts land in mu1_bc/rstd1_bc[:, ts]."""
                mu = vecp.tile([1, tw], BF16, tag="v1")
                ev = vecp.tile([1, tw], F32, tag="v2")
                msq = vecp.tile([1, tw], F32, tag="v3")
                alpha = vecp.tile([1, tw], BF16, tag="v6")
                nc.vector.tensor_scalar_mul(mu[:], ps1[:], 1.0 / D)
                nc.gpsimd.partition_broadcast(mu1_bc[:, ts], mu[0:1, :])
                # mu^2 on ACT (Square is in the restricted table) so the
                # DVE only carries one small op on this path
                nc.scalar.activation(msq[:], ps1[:],
                                     mybir.ActivationFunctionType.Square,
                                     scale=1.0 / D)
                nc.vector.scalar_tensor_tensor(
                    ev[:], ps2[:], 1.0 / D, msq[:], MULT, SUB)
                # r1*r2 = rsqrt((v1+eps)*(v2+eps)) with v2=v1/(v1+eps)
                #       = rsqrt(v1*(1+eps) + eps^2); extra_var folds into
                #       the ln bias: bias = extra*(1+eps) + eps^2
                bias = eps2_sb if extra_var == 0.0 else onep_sb
                nc.scalar.activation(ev[:], ev[:], LN_,
                                     bias=bias[:], scale=1.0 + EPS)
                nc.scalar.activation(alpha[:], ev[:], EXP, scale=-0.5)
                nc.gpsimd.partition_broadcast(rstd1_bc[:, ts], alpha[0:1, :])

            def fused_ln_stats_l0(t0, tw):
                """Layer-0 stats from t / t^2 directly (h = x there)."""
                ts = slice(t0, t0 + tw)
                ps1 = stat_tile(tw)
                for c in range(C):
                    nc.tensor.matmul(ps1[:], ones_bf[:], t_sb[:, c, ts],
                                     start=(c == 0), stop=(c == C - 1))
                ps2 = stat_tile(tw)
                for c in range(C):
                    nc.tensor.matmul(ps2[:], ones_bf[:], tsq_sb[:, c, ts],
                                     start=(c == 0), stop=(c == C - 1))
                fused_ln_alpha(ps1, ps2, tw, ts)

            def fused_ln_stats_a(nq, li):
                """Stats for layers > 0, derived from (r, hf8) only -- no
                dependence on t, so they run interleaved into the NEXT
                step's scores stream. colsum(t) = w2c @ r (LN output h is
                zero-mean); colsum(t^2) = r^T G r + 2 r^T (W2 h) + D (h is
                unit-var): G = W2 W2^T precomputed, W2 h is fp8-DR.
                Part a: mu (+broadcast) and the e = r*(Gr + 2W2h) product."""
                ts = slice(nq * FQ, (nq + 1) * FQ)
                ps1 = stat_tile(FQ)
                nc.tensor.matmul(ps1[:], w2c_sb[:], r_sb[:, ts],
                                 start=True, stop=(li > 0))
                if li == 0:
                    # h = x at layer 0: colsum(h) is the precomputed xsum
                    nc.tensor.matmul(ps1[:], ones_bf[0:1, :],
                                     xsum_sb[0:1, ts],
                                     start=False, stop=True)
                # one accumulation group: psE = 32*(G r + 2 W2 h) -- the
                # host ships g64 = G*32 (bf16) and w2t = W2^T*64 (fp8)
                psE = psA.tile([HID, FQ], F32, tag="main", name="psE")
                nc.tensor.matmul(psE[:], g64_sb[:], r_sb[:, ts],
                                 start=True, stop=False)
                for t2 in range(C // 2):
                    nc.tensor.matmul(
                        psE[:], w2t_sb[:, 2 * t2:2 * t2 + 2, :],
                        hf8_sb[:, 2 * t2:2 * t2 + 2, ts],
                        start=False, stop=(t2 == C // 2 - 1),
                        perf_mode=DR,
                    )
                mu = vecp.tile([1, FQ], BF16, tag="v1")
                msq = vecp.tile([1, FQ], F32, tag="v3")
                nc.vector.tensor_scalar_mul(mu[:], ps1[:], 1.0 / D)
                nc.gpsimd.partition_broadcast(mu1_bc[:, ts], mu[0:1, :])
                nc.scalar.activation(msq[:], ps1[:],
                                     mybir.ActivationFunctionType.Square,
                                     scale=1.0 / D)
                nc.vector.scalar_tensor_tensor(
                    e_sb[:, ts], psE[:], 1.0 / (WS / 2.0), r_sb[:, ts],
                    MULT, MULT)
                return msq

            def fused_ln_stats_b(nq, li, msq):
                """Part b: colsum(e) -> alpha chain -> rstd broadcast.
                Layer 0 accumulates the exact precomputed sum(x^2) instead
                of relying on the unit-variance identity."""
                ts = slice(nq * FQ, (nq + 1) * FQ)
                ps2 = stat_tile(FQ)
                nc.tensor.matmul(ps2[:], ones_bf[:HID, :], e_sb[:, ts],
                                 start=True, stop=(li > 0))
                if li == 0:
                    nc.tensor.matmul(ps2[:], ones_bf[0:1, :],
                                     xsqs_sb[0:1, ts],
                                     start=False, stop=True)
                ev = vecp.tile([1, FQ], F32, tag="v2")
                alpha = vecp.tile([1, FQ], BF16, tag="v6")
                nc.vector.scalar_tensor_tensor(
                    ev[:], ps2[:], 1.0 / D, msq[:], MULT, SUB)
                nc.scalar.activation(ev[:], ev[:], LN_,
                                     bias=eps2_sb[:] if li == 0 else onep_sb[:],
                                     scale=1.0 + EPS)
                nc.scalar.activation(alpha[:], ev[:], EXP, scale=-0.5)
                nc.gpsimd.partition_broadcast(rstd1_bc[:, ts], alpha[0:1, :])

            def fused_ln_norm(t0, tw, out_last=False):
                """Apply h = (t - mu)*alpha using the precomputed
                broadcasts. hf8 (chunks 0-1 first) gates the next layer's
                scores, so it is written before the bf16 h."""
                ts = slice(t0, t0 + tw)
                bshape = (P, C, tw)
                bs2 = (P, 2, tw)
                nc.vector.tensor_tensor(
                    h_sb[:, :, ts], t_sb[:, :, ts],
                    mu1_bc[:, None, ts].to_broadcast(bshape), SUB,
                )
                if out_last:
                    nc.vector.tensor_tensor(
                        hout_sb[:, :, ts], h_sb[:, :, ts],
                        rstd1_bc[:, None, ts].to_broadcast(bshape), MULT,
                    )
                    store_eng = [nc.sync, nc.scalar, nc.gpsimd, nc.sync]
                    for c in range(C):
                        store_eng[c].dma_start(out[c, :, ts],
                                               hout_sb[:, c, ts])
                else:
                    # hf8 chunk-pair 0-1 first: the next layer's first
                    # scores matmul needs only those chunks
                    for h2 in range(2):
                        nc.vector.tensor_tensor(
                            hf8_sb[:, 2 * h2:2 * h2 + 2, ts],
                            h_sb[:, 2 * h2:2 * h2 + 2, ts],
                            rstd1_bc[:, None, ts].to_broadcast(bs2), MULT,
                        )
                    nc.vector.tensor_tensor(
                        h_sb[:, :, ts], h_sb[:, :, ts],
                        rstd1_bc[:, None, ts].to_broadcast(bshape), MULT,
                    )

            # ---- transformer layers: flat chunk-step pipeline ----
            steps = [(li, nq) for li in range(L) for nq in range(NQ)]
            pending_ln = []

            fast = trivial_bias and trivial_ln
            emit_scores(steps[0][1])
            # per-token sum(x), sum(x^2) for the layer-0 LN stats; sits
            # behind the first scores block so the late h DMA is covered
            if fast:
                for nqq in range(NQ):
                    tsx = slice(nqq * FQ, (nqq + 1) * FQ)
                    nc.vector.tensor_mul(tsq_sb[:, :, tsx], h_sb[:, :, tsx],
                                         h_sb[:, :, tsx])
                    psx = stat_tile(FQ)
                    for c in range(C):
                        nc.tensor.matmul(psx[:], ones_bf[:], h_sb[:, c, tsx],
                                         start=(c == 0), stop=(c == C - 1))
                    nc.vector.tensor_copy(xsum_sb[0:1, tsx], psx[:])
                    psx2 = stat_tile(FQ)
                    for c in range(C):
                        nc.tensor.matmul(psx2[:], ones_bf[:],
                                         tsq_sb[:, c, tsx],
                                         start=(c == 0), stop=(c == C - 1))
                    nc.vector.tensor_copy(xsqs_sb[0:1, tsx], psx2[:])
            for i, (li, nq) in enumerate(steps):
                last = li == L - 1
                # previous step's pending work flushes here: the layer-0 LN
                # (or general-path LN) DVE chain hides under attn+scores
                while pending_ln:
                    pending_ln.pop(0)()

                if fast:
                    # merged steady-state: next step's scores groups
                    # interleave with this step's attn chunks so the exp
                    # chain starts ~4us earlier and attn matmuls fill its
                    # wait slots; LN stats (f(r, hf8) only) and ff1 slot
                    # into the scores tail
                    final = i == len(steps) - 1
                    if not final:
                        for u in range(4):
                            attn_chunk(nq, u)
                        emit_ff1(nq)
                        nxt = steps[i + 1][1]
                        psd = scores_begin()
                        msq = None
                        for g in range(MK // 2):
                            scores_group(nxt, g, psd)
                            if g == 2:
                                msq = fused_ln_stats_a(nq, li)
                            elif g == 5:
                                fused_ln_stats_b(nq, li, msq)
                        scores_end(nxt, psd)
                        emit_ff2(nq, li, inline_norm=True, out_last=last)
                    else:
                        for u in range(4):
                            attn_chunk(nq, u)
                        emit_ff1(nq)
                        msq = fused_ln_stats_a(nq, li)
                        fused_ln_stats_b(nq, li, msq)
                        # final step: ff2 + normalize + store in
                        # half-windows so the tail drains incrementally
                        sw = FQ // 2
                        for j in range(2):
                            emit_ff2(nq, li, nq * FQ + j * sw, sw,
                                     inline_norm=True, out_last=True)
                    continue

                emit_attn(nq)
                emit_ff1(nq)
                if i + 1 < len(steps):
                    emit_scores(steps[i + 1][1])
                emit_ff2(nq, li)

                def _ln(nq=nq, last=last, li=li):
                    if trivial_ln:
                        fused_ln_stats_l0(nq * FQ, FQ)
                        fused_ln_norm(nq * FQ, FQ, out_last=last)
                    else:
                        layer_norm(t_sb, t_sb, g1_sb, be1_sb,
                                   mu1_bc, rstd1_bc, nq, use_stw=True)
                        layer_norm(t_sb, h_sb, g2_sb, be2_sb,
                                   mu2_bc, rstd2_bc, nq, out_last=last)
                pending_ln.append(_ln)
            while pending_ln:
                pending_ln.pop(0)()
    nc.compile()
    return nc


_NC_CACHE = {}


def _get_nc(trivial_ln, trivial_bias=False):
    key = ("nc", trivial_ln, trivial_bias)
    if key not in _NC_CACHE:
        _NC_CACHE[key] = build(trivial_ln=trivial_ln, trivial_bias=trivial_bias)
    return _NC_CACHE[key]


def _shard_inputs(x, Wq, bq, Wk, bk_, Wv, bv_, W1, b1, W2, b2, ln1_g, ln1_b, ln2_g, ln2_b):
    """Full inputs -> list of 8 per-core in_maps."""
    bf = ml_dtypes.bfloat16
    f8 = ml_dtypes.float8_e4m3
    C = D // P
    SQ = S // 2
    shared = {
        "wq": np.ascontiguousarray(Wq.T * WS).reshape(C, P, D).astype(f8),
        "wk": np.ascontiguousarray(Wk * WS).reshape(C, P, D).astype(f8),
        "wv": np.ascontiguousarray(Wv * WS).reshape(C, P, D).astype(f8),
        "w1": np.ascontiguousarray(W1 * WS).reshape(C, P, HID).astype(f8),
        "w2": np.ascontiguousarray(W2).astype(bf),
        "w2c": np.ascontiguousarray(W2.sum(axis=1).reshape(HID, 1)).astype(bf),
        "w2t": np.ascontiguousarray(W2.T * WS).reshape(C, P, HID).astype(f8),
        "g64": np.ascontiguousarray((W2 @ W2.T) * (WS / 2.0)).astype(bf),
        "bqc": np.ascontiguousarray(bq.reshape(C, P, 1)).astype(f8),
        "bk": np.ascontiguousarray(bk_.reshape(C, P).T).astype(np.float32),
        "bv": np.ascontiguousarray(bv_.reshape(1, D) * VS).astype(np.float32),
        "b1d": np.ascontiguousarray(b1.reshape(HID, 1)).astype(np.float32),
        "b2r": np.ascontiguousarray(b2.reshape(1, D)).astype(bf),
        "g1d": np.ascontiguousarray(ln1_g.reshape(C, P).T).astype(np.float32),
        "be1d": np.ascontiguousarray(ln1_b.reshape(C, P).T).astype(np.float32),
        "g2d": np.ascontiguousarray(ln2_g.reshape(C, P).T).astype(np.float32),
        "be2d": np.ascontiguousarray(ln2_b.reshape(C, P).T).astype(np.float32),
    }
    in_maps = []
    for core in range(8):
        b, j = core // 2, core % 2
        xT = np.ascontiguousarray(x[b].T)  # [D, S]
        xT8 = xT.astype(f8)
        m = dict(shared)
        m["xt"] = xT8.reshape(C, P, S)
        m["xq"] = np.ascontiguousarray(
            xT[:, j * SQ:(j + 1) * SQ].reshape(C, P, SQ)
        ).astype(bf)
        m["xq8"] = np.ascontiguousarray(
            xT8.reshape(C, P, S)[:, :, j * SQ:(j + 1) * SQ])
        in_maps.append(m)
    return in_maps


def _gather_output(results):
    SQ = S // 2
    out = np.empty((B, S, D), np.float32)
    for core, res in enumerate(results):
        b, j = core // 2, core % 2
        # res["out"]: [C, P, SQ] = h^T chunks -> h slice [SQ, D]
        out[b, j * SQ:(j + 1) * SQ, :] = (
            res["out"].astype(np.float32).reshape(D, SQ).T)
    return out


def _ln_trivial(inputs):
    return bool(
        np.all(inputs["ln1_g"] == 1.0) and np.all(inputs["ln1_b"] == 0.0)
        and np.all(inputs["ln2_g"] == 1.0) and np.all(inputs["ln2_b"] == 0.0)
    )


def _bias_trivial(inputs):
    return bool(all(np.all(inputs[k] == 0.0) for k in ("bq", "b2", "bv")))


def kernel(**inputs):
    nc = _get_nc(trivial_ln=_ln_trivial(inputs), trivial_bias=_bias_trivial(inputs))
    in_maps = _shard_inputs(
        inputs["x"], inputs["Wq"], inputs["bq"], inputs["Wk"], inputs["bk"],
        inputs["Wv"], inputs["bv"], inputs["W1"], inputs["b1"], inputs["W2"],
        inputs["b2"], inputs["ln1_g"], inputs["ln1_b"], inputs["ln2_g"],
        inputs["ln2_b"],
    )
    res = run_bass_kernel_spmd(nc, in_maps, core_ids=list(range(8)))
    return _gather_output(res.results)
